# revision 26
# baseline (speedup 1.0000x reference)
"""CP(n) lattice action kernel for Trainium2 (8 NeuronCores, Bass/Tile).

Fast path for the roll-structured shift (nearest-neighbor on the 64x64
lattice); general gather-based fallback for arbitrary shift tables.
See _build_fast for the layout/math notes.
"""
import contextlib
import sys

import numpy as np

sys.path.insert(0, "/opt/trn_rl_repo")

B, S, NA = 1024, 4096, 6
NCORES = 8
PB = B // NCORES          # 128 batches per core
L = 64                    # lattice row length
NROW = S // L             # 64 rows
ROWP = L + 1              # padded row length
XLEN = NROW * ROWP + ROWP  # 4225: 64 padded rows + wrap row
PI = float(np.pi)
NBETA = 4.0               # N * BETA


_cache = {}


def _detect_roll(shift):
    idx = np.arange(S).reshape(L, L)
    s0 = np.roll(idx, -1, axis=0).ravel()
    s1 = np.roll(idx, -1, axis=1).ravel()
    return np.array_equal(shift[0], s0) and np.array_equal(shift[1], s1)


def _runs(perm):
    runs = []
    st = 0
    for i in range(1, len(perm) + 1):
        if i == len(perm) or perm[i] != perm[i - 1] + 1:
            runs.append((st, int(perm[st]), i - st))
            st = i
    return runs


FAST_V2 = True


def _build(shift, reps=1, mode="full", dummy_io=False):
    if _detect_roll(shift):
        if FAST_V2:
            return _build_fast2(reps=reps, mode=mode, dummy_io=dummy_io)
        return _build_fast(reps=reps, mode=mode, dummy_io=dummy_io)
    return _build_general(shift, reps=reps, mode=mode)


GC1 = 2048
GN1 = S // GC1
GC2 = 2048
GN2 = S // GC2


def _build_general(shift, reps=1, mode="full"):
    import concourse.bass as bass
    import concourse.tile as tile
    from concourse import bacc, mybir

    f32 = mybir.dt.float32
    bf16 = mybir.dt.bfloat16
    i16 = mybir.dt.int16
    Act = mybir.ActivationFunctionType
    Op = mybir.AluOpType
    X = mybir.AxisListType.X

    roll = _detect_roll(shift)

    nc = bacc.Bacc(None, target_bir_lowering=False)
    phi_d = nc.dram_tensor("phi", [PB, S, NA], f32, kind="ExternalInput")
    out_d = nc.dram_tensor("out", [PB, 1], f32, kind="ExternalOutput")
    pd_flat = phi_d[:].rearrange("p s a -> p (s a)")

    with tile.TileContext(nc) as tc:
        with contextlib.ExitStack() as ctx:
            xfull_pool = ctx.enter_context(tc.tile_pool(name="xfull", bufs=1))
            small_pool = ctx.enter_context(tc.tile_pool(name="small", bufs=1))

            NACC = GN2
            acc = small_pool.tile([PB, NACC], f32)
            if roll:
                xf = xfull_pool.tile([PB, 7, S], bf16)
                xg = None
            else:
                # site-major cells [site, 8] so gpsimd.ap_gather can fetch
                # whole 7-component cells per shift index
                xf = None
                xg = xfull_pool.tile([PB, S, 8], bf16)
                idx_sb = []
                for d in range(2):
                    wrapped = np.zeros((PB, S // 16), np.int16)
                    base = shift[d].reshape(S // 16, 16).T.astype(np.int16)
                    for g in range(PB // 16):
                        wrapped[16 * g:16 * (g + 1)] = base
                    hdl = nc.inline_tensor(wrapped, name=f"shift_idx_{d}")
                    t_ = small_pool.tile([PB, S // 16], mybir.dt.int16, tag=f"idx{d}")
                    nc.sync.dma_start(t_[:], hdl[:])
                    idx_sb.append(t_)

            for rep in range(reps):
                # ======== stage 1: wrap + trig + embedding ========
                st1 = contextlib.ExitStack()
                p_phi = st1.enter_context(tc.tile_pool(name="p_phi", bufs=1))
                p_ks = st1.enter_context(tc.tile_pool(name="p_ks", bufs=1))
                p_t = st1.enter_context(tc.tile_pool(name="p_t", bufs=1))
                p_u = st1.enter_context(tc.tile_pool(name="p_u", bufs=1))
                p_cum = st1.enter_context(tc.tile_pool(name="p_cum", bufs=1))

                for ch in range(GN1):
                    cs = ch * GC1
                    M = GC1 * NA

                    phic = p_phi.tile([PB, M], f32, tag="phic")
                    nc.sync.dma_start(phic[:], pd_flat[:, cs * NA:(cs + GC1) * NA])

                    if mode == "dma":
                        nc.vector.tensor_reduce(acc[:, 0:1], phic[:, 0:8],
                                                axis=X, op=Op.add)
                        continue

                    # k = round(phi/2pi) as int16
                    k = p_ks.tile([PB, M], i16, tag="ks")
                    nc.vector.tensor_scalar(k[:], phic[:], 1.0 / (2 * PI), None,
                                            op0=Op.mult)
                    # phir = (k * -2pi) + phi   (in place)
                    nc.vector.scalar_tensor_tensor(
                        phic[:], k[:], -2 * PI, phic[:], op0=Op.mult, op1=Op.add)

                    # sigma, t = Sin(phir)  (interleaved site-major, bf16)
                    sig = p_ks.tile([PB, M], bf16, tag="ks")
                    nc.scalar.activation(sig[:], phic[:], Act.Sign)
                    tt = p_t.tile([PB, M], bf16, tag="t")
                    nc.scalar.activation(tt[:], phic[:], Act.Sin)
                    # arg2 = sigma*pi/2 - phir (in place over phir)
                    nc.vector.scalar_tensor_tensor(
                        phic[:], sig[:], PI / 2, phic[:],
                        op0=Op.mult, op1=Op.subtract)
                    # u = Sin(arg2) = sigma*cos(phir)
                    uu = p_u.tile([PB, M], bf16, tag="u")
                    nc.scalar.activation(uu[:], phic[:], Act.Sin)

                    def ang(tile_, j, n=1):
                        ap = tile_[:]
                        if n == 1:
                            return bass.AP(tensor=ap.tensor, offset=ap.offset + j,
                                           ap=[ap.ap[0], [NA, GC1]])
                        return bass.AP(tensor=ap.tensor, offset=ap.offset + j,
                                       ap=[ap.ap[0], [NA, GC1], [1, n]])

                    # s_j = sigma*t for j<5 (in place on t)
                    nc.vector.tensor_tensor(ang(tt, 0, 5), ang(tt, 0, 5),
                                            ang(sig, 0, 5), op=Op.mult)
                    # c_5 = sigma*u at j=5 (in place on u)
                    nc.vector.tensor_tensor(ang(uu, 5), ang(uu, 5),
                                            ang(sig, 5), op=Op.mult)

                    # cumprod + x build into xf rows / xg cells
                    cumA = p_cum.tile([PB, GC1], bf16, tag="cumA")
                    cumB = p_cum.tile([PB, GC1], bf16, tag="cumB")
                    if roll:
                        xs = xf[:, :, cs:cs + GC1]
                        xk = [xs[:, k, :] for k in range(7)]
                    else:
                        gap = xg[:]
                        xk = [bass.AP(tensor=gap.tensor,
                                      offset=gap.offset + cs * 8 + k,
                                      ap=[gap.ap[0], [8, GC1]])
                              for k in range(7)]
                    nc.vector.tensor_copy(xk[0], ang(uu, 0))
                    nc.vector.tensor_tensor(xk[1], ang(uu, 1), ang(tt, 0),
                                            op=Op.mult)
                    nc.vector.tensor_tensor(cumA[:], ang(tt, 0), ang(tt, 1),
                                            op=Op.mult)
                    nc.vector.tensor_tensor(xk[2], ang(uu, 2), cumA[:],
                                            op=Op.mult)
                    nc.vector.tensor_tensor(cumB[:], cumA[:], ang(tt, 2),
                                            op=Op.mult)
                    nc.vector.tensor_tensor(xk[3], ang(uu, 3), cumB[:],
                                            op=Op.mult)
                    nc.vector.tensor_tensor(cumA[:], cumB[:], ang(tt, 3),
                                            op=Op.mult)
                    nc.vector.tensor_tensor(xk[4], ang(uu, 4), cumA[:],
                                            op=Op.mult)
                    nc.vector.tensor_tensor(cumB[:], cumA[:], ang(tt, 4),
                                            op=Op.mult)
                    nc.vector.tensor_tensor(xk[5], ang(uu, 5), cumB[:],
                                            op=Op.mult)
                    nc.vector.tensor_tensor(xk[6], cumB[:], ang(tt, 5),
                                            op=Op.mult)

                st1.close()
                if mode in ("dma", "stage1"):
                    continue

                # ======== stage 2: neighbor products ========
                st2 = contextlib.ExitStack()
                p_xp = st2.enter_context(tc.tile_pool(name="p_xp", bufs=1))
                p_m = st2.enter_context(tc.tile_pool(name="p_m", bufs=1))
                p_pq = st2.enter_context(tc.tile_pool(name="p_pq", bufs=1))

                for ch in range(GN2):
                    cs = ch * GC2
                    if roll:
                        xs = xf[:, :, cs:cs + GC2]
                    else:
                        gap = xg[:]
                        xs = None
                        xg_k = lambda k0, n, off=0: bass.AP(
                            tensor=gap.tensor,
                            offset=gap.offset + cs * 8 + k0,
                            ap=[gap.ap[0], [1, n], [8, GC2]])

                    # double-width: both dirs side by side, shared folds
                    m = p_m.tile([PB, 7, 2 * GC2], bf16, tag="m")
                    pq = p_pq.tile([PB, 6, 2 * GC2], bf16, tag="pq")

                    for d in (0, 1):
                        if roll and d == 0:
                            lo = cs + L
                            if lo + GC2 <= S:
                                xp_ap = xf[:, :, lo:lo + GC2]
                            else:
                                xp = p_xp.tile([PB, 7, GC2], bf16, tag="xp")
                                mn = S - lo
                                nc.vector.tensor_copy(xp[:, :, 0:mn],
                                                      xf[:, :, lo:S])
                                nc.vector.tensor_copy(xp[:, :, mn:GC2],
                                                      xf[:, :, 0:GC2 - mn])
                                xp_ap = xp[:]
                        elif roll and d == 1:
                            xp = p_xp.tile([PB, 7, GC2], bf16, tag="xp")
                            nrow = GC2 // L
                            src = bass.AP(
                                tensor=xf.tensor, offset=xf[:].offset + cs + 1,
                                ap=[xf[:].ap[0], [S, 7], [L, nrow], [1, L - 1]])
                            dst = bass.AP(
                                tensor=xp.tensor, offset=xp[:].offset,
                                ap=[xp[:].ap[0], [GC2, 7], [L, nrow], [1, L - 1]])
                            nc.gpsimd.tensor_copy(dst, src)
                            srcw = bass.AP(
                                tensor=xf.tensor, offset=xf[:].offset + cs,
                                ap=[xf[:].ap[0], [S, 7], [L, nrow]])
                            dstw = bass.AP(
                                tensor=xp.tensor, offset=xp[:].offset + L - 1,
                                ap=[xp[:].ap[0], [GC2, 7], [L, nrow]])
                            nc.gpsimd.tensor_copy(dstw, srcw)
                            xp_ap = xp[:]
                        else:
                            xpg = p_xp.tile([PB, GC2, 8], bf16, tag="xp")
                            nc.gpsimd.ap_gather(
                                xpg[:], xg[:],
                                idx_sb[d][:, cs // 16:(cs + GC2) // 16],
                                channels=PB, num_elems=S, d=8, num_idxs=GC2)
                            gp = xpg[:]
                            xp_k = lambda k0, n: bass.AP(
                                tensor=gp.tensor, offset=gp.offset + k0,
                                ap=[gp.ap[0], [1, n], [8, GC2]])

                        ms = m[:, :, d * GC2:(d + 1) * GC2]
                        pqs = pq[:, :, d * GC2:(d + 1) * GC2]
                        if roll:
                            nc.vector.tensor_tensor(ms, xs, xp_ap, op=Op.mult)
                            nc.vector.tensor_tensor(
                                pqs[:, 0:3, :], xs[:, 0:3, :],
                                xp_ap[:, 4:7, :], op=Op.mult)
                            nc.vector.tensor_tensor(
                                pqs[:, 3:6, :], xs[:, 4:7, :],
                                xp_ap[:, 0:3, :], op=Op.mult)
                        else:
                            nc.vector.tensor_tensor(ms, xg_k(0, 7),
                                                    xp_k(0, 7), op=Op.mult)
                            nc.vector.tensor_tensor(pqs[:, 0:3, :], xg_k(0, 3),
                                                    xp_k(4, 3), op=Op.mult)
                            nc.vector.tensor_tensor(pqs[:, 3:6, :], xg_k(4, 3),
                                                    xp_k(0, 3), op=Op.mult)

                    # shared folds across both dirs (double width)
                    nc.vector.tensor_tensor(m[:, 0:3, :], m[:, 0:3, :],
                                            m[:, 4:7, :], op=Op.subtract)
                    nc.vector.tensor_tensor(m[:, 0:2, :], m[:, 0:2, :],
                                            m[:, 2:4, :], op=Op.add)
                    nc.vector.tensor_tensor(m[:, 0, :], m[:, 0, :],
                                            m[:, 1, :], op=Op.add)
                    nc.vector.tensor_tensor(pq[:, 0:3, :], pq[:, 0:3, :],
                                            pq[:, 3:6, :], op=Op.add)
                    nc.vector.tensor_tensor(pq[:, 0, :], pq[:, 0, :],
                                            pq[:, 1, :], op=Op.add)
                    nc.vector.tensor_tensor(m[:, 1, :], pq[:, 0, :],
                                            pq[:, 2, :], op=Op.add)

                    # one fused square+accum per chunk (wr,wi x both dirs),
                    # squared in place over the fold results
                    nc.scalar.activation(m[:, 0:2, :], m[:, 0:2, :], Act.Square,
                                         accum_out=acc[:, ch:ch + 1])

                st2.close()

            # ======== final reduce + affine ========
            stot = small_pool.tile([PB, 1], f32)
            nc.vector.tensor_reduce(stot[:], acc[:], axis=X, op=Op.add)
            res = small_pool.tile([PB, 1], f32)
            nc.vector.tensor_scalar(res[:], stot[:], -NBETA, NBETA * 2.0 * S,
                                    op0=Op.mult, op1=Op.add)
            nc.sync.dma_start(out_d[:], res[:])

    nc.finalize()
    return nc



def _build_fast(reps=1, mode="full", dummy_io=False):
    """Roll-shift fast path, instruction-count-minimal.

    This runtime is per-instruction-overhead dominated (~40-100us per
    instruction regardless of size), so the kernel uses FEW, HUGE
    instructions: 2 stage-1 chunks of 2048 sites, and a single full-S
    pass per direction in stage 2 (~65 instructions per rep vs ~75 for
    the previous baseline).

    Layout: x embedding stored angle-major in a 65-padded site layout
    (site (r,c) at p=65r+c; col 64 of each row = col 0 copy; row 64 =
    row 0 copy), so both lattice neighbor shifts are pure offset views
    (+1 for d=1, +65 for d=0) -- no gather/shift copies at all.

    Math (identical to the proven baseline): wrap via k=RNE(phi/2pi),
    phir=phi-2pi*k; sigma=Sign(phir); s_j=sigma*Sin(phir) (j<5),
    c_j=Sin(sigma*pi/2-phir)=sigma*cos(phir) (c_5 fixed up by sigma);
    hyperspherical embedding x via cumprod; per dir:
    m_k=x_k x'_k, pq = zr*zi' / zi*zr'; dre/dim folds; ACT Square+accum.
    """
    import concourse.bass as bass
    import concourse.tile as tile
    from concourse import bacc, mybir

    f32 = mybir.dt.float32
    bf16 = mybir.dt.bfloat16
    i16 = mybir.dt.int16
    Act = mybir.ActivationFunctionType
    Op = mybir.AluOpType
    X = mybir.AxisListType.X

    FC = 2048                 # stage-1 chunk (sites)
    FN = S // FC
    FR = FC // L              # rows per chunk
    M = FC * NA

    nc = bacc.Bacc(None, target_bir_lowering=False)
    phi_kind = "Internal" if dummy_io else "ExternalInput"
    phi_d = nc.dram_tensor("phi", [PB, S, NA], f32, kind=phi_kind)
    out_d = nc.dram_tensor("out", [PB, 1], f32, kind="ExternalOutput")
    pd_flat = phi_d[:].rearrange("p s a -> p (s a)")

    def ap(tile_, off, dims):
        base = tile_[:]
        return bass.AP(tensor=base.tensor, offset=base.offset + off,
                       ap=[base.ap[0]] + dims)

    with tile.TileContext(nc) as tc:
        with contextlib.ExitStack() as ctx:
            xp = ctx.enter_context(tc.tile_pool(name="xp", bufs=1))
            sp = ctx.enter_context(tc.tile_pool(name="sp", bufs=1))

            xf = xp.tile([PB, 7, XLEN], bf16)
            acc = sp.tile([PB, max(reps, 1)], f32)

            for rep in range(reps):
                st1 = contextlib.ExitStack()
                p_phi = st1.enter_context(tc.tile_pool(name="p_phi", bufs=1))
                p_ks = st1.enter_context(tc.tile_pool(name="p_ks", bufs=1))
                p_t = st1.enter_context(tc.tile_pool(name="p_t", bufs=1))
                p_u = st1.enter_context(tc.tile_pool(name="p_u", bufs=1))
                p_cum = st1.enter_context(tc.tile_pool(name="p_cum", bufs=1))

                for ch in range(FN):
                    cs = ch * FC
                    poff = (cs // L) * ROWP

                    phic = p_phi.tile([PB, M], f32, tag="phic")
                    nc.sync.dma_start(phic[:], pd_flat[:, cs * NA:(cs + FC) * NA])
                    if mode == "dma":
                        nc.vector.tensor_reduce(acc[:, 0:1], phic[:, 0:8],
                                                axis=X, op=Op.add)
                        continue

                    # k = RNE(phi/2pi); phir = phi - 2pi*k (in place)
                    k = p_ks.tile([PB, M], i16, tag="ks")
                    nc.vector.tensor_scalar(k[:], phic[:], 1.0 / (2 * PI), None,
                                            op0=Op.mult)
                    nc.vector.scalar_tensor_tensor(
                        phic[:], k[:], -2 * PI, phic[:], op0=Op.mult, op1=Op.add)

                    sig = p_ks.tile([PB, M], bf16, tag="ks")
                    nc.scalar.activation(sig[:], phic[:], Act.Sign)
                    tt_ = p_t.tile([PB, M], bf16, tag="t")
                    nc.scalar.activation(tt_[:], phic[:], Act.Sin)
                    # arg2 = sigma*pi/2 - phir (in place); u = Sin(arg2)
                    nc.vector.scalar_tensor_tensor(
                        phic[:], sig[:], PI / 2, phic[:],
                        op0=Op.mult, op1=Op.subtract)
                    uu = p_u.tile([PB, M], bf16, tag="u")
                    nc.scalar.activation(uu[:], phic[:], Act.Sin)

                    def ang(tile_, j, n=1):
                        a_ = tile_[:]
                        if n == 1:
                            return bass.AP(tensor=a_.tensor,
                                           offset=a_.offset + j,
                                           ap=[a_.ap[0], [NA, FC]])
                        return bass.AP(tensor=a_.tensor, offset=a_.offset + j,
                                       ap=[a_.ap[0], [NA, FC], [1, n]])

                    # s_j = sigma*t (j<5); c_5 = sigma*u (in place)
                    nc.vector.tensor_tensor(ang(tt_, 0, 5), ang(tt_, 0, 5),
                                            ang(sig, 0, 5), op=Op.mult)
                    nc.vector.tensor_tensor(ang(uu, 5), ang(uu, 5),
                                            ang(sig, 5), op=Op.mult)

                    # cumprod + x build into padded xf rows
                    cumA = p_cum.tile([PB, FC], bf16, tag="cumA")
                    cumB = p_cum.tile([PB, FC], bf16, tag="cumB")

                    def xrow(kk):
                        return ap(xf, kk * XLEN + poff, [[ROWP, FR], [1, L]])

                    def angr(tile_, j):
                        a_ = tile_[:]
                        return bass.AP(tensor=a_.tensor, offset=a_.offset + j,
                                       ap=[a_.ap[0], [NA * L, FR], [NA, L]])

                    flat = [[L, FR], [1, L]]
                    TT = nc.vector.tensor_tensor
                    nc.vector.tensor_copy(xrow(0), angr(uu, 0))
                    TT(xrow(1), angr(uu, 1), angr(tt_, 0), op=Op.mult)
                    TT(ap(cumA, 0, flat), angr(tt_, 0), angr(tt_, 1), op=Op.mult)
                    TT(xrow(2), angr(uu, 2), ap(cumA, 0, flat), op=Op.mult)
                    TT(ap(cumB, 0, flat), ap(cumA, 0, flat), angr(tt_, 2),
                       op=Op.mult)
                    TT(xrow(3), angr(uu, 3), ap(cumB, 0, flat), op=Op.mult)
                    TT(ap(cumA, 0, flat), ap(cumB, 0, flat), angr(tt_, 3),
                       op=Op.mult)
                    TT(xrow(4), angr(uu, 4), ap(cumA, 0, flat), op=Op.mult)
                    TT(ap(cumB, 0, flat), ap(cumA, 0, flat), angr(tt_, 4),
                       op=Op.mult)
                    TT(xrow(5), angr(uu, 5), ap(cumB, 0, flat), op=Op.mult)
                    TT(xrow(6), ap(cumB, 0, flat), angr(tt_, 5), op=Op.mult)

                # col pads for all rows at once, then the wrap row
                nc.scalar.copy(
                    ap(xf, L, [[XLEN, 7], [ROWP, NROW]]),
                    ap(xf, 0, [[XLEN, 7], [ROWP, NROW]]))
                nc.scalar.copy(
                    ap(xf, NROW * ROWP, [[XLEN, 7], [1, ROWP]]),
                    ap(xf, 0, [[XLEN, 7], [1, ROWP]]))

                st1.close()
                if mode in ("dma", "stage1"):
                    continue

                # ======== stage 2: one full-S pass per direction ========
                st2 = contextlib.ExitStack()
                p_mt = st2.enter_context(tc.tile_pool(name="p_mt", bufs=1))
                # dir 1 uses rows shifted by +2 so both dirs' dre/dim
                # land in contiguous rows 0-3 and share one Square+accum
                mt = p_mt.tile([PB, 15, S], bf16, tag="mt")

                NR = S // L

                TT = nc.vector.tensor_tensor
                for d, off in ((0, ROWP), (1, 1)):
                    r0 = 2 * d

                    def mrow(i, n=1):
                        return ap(mt, (r0 + i) * S, [[S, n], [L, NR], [1, L]])

                    def mfl(i, n=1):
                        return ap(mt, (r0 + i) * S, [[S, n], [1, S]])

                    def xv(k0, n, o):
                        return ap(xf, k0 * XLEN + o,
                                  [[XLEN, n], [ROWP, NR], [1, L]])

                    # m_k rows 0-6; pq rows 7-9 (zr*zi') and 10-12 (zi*zr')
                    TT(mrow(0, 7), xv(0, 7, 0), xv(0, 7, off), op=Op.mult)
                    TT(mrow(7, 3), xv(0, 3, 0), xv(4, 3, off), op=Op.mult)
                    TT(mrow(10, 3), xv(4, 3, 0), xv(0, 3, off), op=Op.mult)
                    # dre folds: m[0:3]-=m[4:7]; m[0:2]+=m[2:4]
                    TT(mfl(0, 3), mfl(0, 3), mfl(4, 3), op=Op.subtract)
                    TT(mfl(0, 2), mfl(0, 2), mfl(2, 2), op=Op.add)
                    # dim partial: pq[7:10]+=pq[10:13]
                    TT(mfl(7, 3), mfl(7, 3), mfl(10, 3), op=Op.add)
                    # fused: m0+=m1 AND pq7+=pq8 (rows {0,7} += rows {1,8})
                    def mpair(i):
                        return ap(mt, (r0 + i) * S, [[7 * S, 2], [1, S]])
                    TT(mpair(0), mpair(0), mpair(1), op=Op.add)
                    # dim finish: m1 = pq7 + pq9
                    TT(mfl(1), mfl(7), mfl(9), op=Op.add)
                # one Square+accum over rows 0-3 (dre0, dim0, dre1, dim1)
                nc.scalar.activation(
                    ap(mt, 0, [[S, 4], [1, S]]),
                    ap(mt, 0, [[S, 4], [1, S]]),
                    Act.Square, accum_out=acc[:, rep:rep + 1])
                st2.close()

            # ======== final reduce + affine ========
            stot = sp.tile([PB, 1], f32)
            nc.vector.tensor_reduce(stot[:], acc[:, reps - 1:reps],
                                    axis=X, op=Op.add)
            res = sp.tile([PB, 1], f32)
            nc.vector.tensor_scalar(res[:], stot[:], -NBETA, NBETA * 2.0 * S,
                                    op0=Op.mult, op1=Op.add)
            nc.sync.dma_start(out_d[:], res[:])

    nc.finalize()
    return nc


def _build_fast2(reps=1, mode="full", dummy_io=False,
                 pool_folds=False, cp_act=True,
                 phr_bufs=2, mt_bufs=2, php_bufs=1, wrap_arw=True,
                 abs_cos=True):
    """Restructured roll-shift fast path (v2).

    vs _build_fast: angle-major (de-interleaved) trig outputs so all DVE
    ops are row-contiguous; pair-tree cumprod (4 ops instead of a 5-long
    serial chain); both lattice directions packed side by side in each
    stage-2 row so every fold/square instruction covers them at once;
    copies/pads and two of the folds moved to the otherwise-idle gpsimd
    engine; phi pool double-buffered so chunk DMAs overlap compute.

    Math identical to _build_fast: wrap via k=RNE(phi/2pi),
    phir=phi-2pi*k; sigma=Sign(phir); s~=sigma*sin(phir) (|sin|, j<5);
    u=Sin(sigma*pi/2-phir)=sigma*cos(phir) (c5 fixed by extra sigma5);
    hyperspherical x via cumprod tree; per dir m_k/pq products; fold
    tree; Square+accum.
    """
    import concourse.bass as bass
    import concourse.tile as tile
    from concourse import bacc, mybir

    f32 = mybir.dt.float32
    bf16 = mybir.dt.bfloat16
    i16 = mybir.dt.int16
    Act = mybir.ActivationFunctionType
    Op = mybir.AluOpType
    X = mybir.AxisListType.X

    FC = 1024                 # stage-1 chunk (sites)
    CN = S // FC              # 4 chunks
    FR = FC // L              # 16 lattice rows per chunk
    M = FC * NA               # 6144 angles per chunk
    QS = 1024                 # stage-2 quarter (sites per dir)
    NQ = S // QS
    NRQ = QS // L             # 16 lattice rows per quarter
    DS = 2 * QS               # both dirs side by side

    nc = bacc.Bacc(None, target_bir_lowering=False)
    phi_kind = "Internal" if dummy_io else "ExternalInput"
    phi_d = nc.dram_tensor("phi", [PB, S, NA], f32, kind=phi_kind)
    out_d = nc.dram_tensor("out", [PB, 1], f32, kind="ExternalOutput")
    pd_flat = phi_d[:].rearrange("p s a -> p (s a)")

    def ap(tile_, off, dims):
        base = tile_[:]
        return bass.AP(tensor=base.tensor, offset=base.offset + off,
                       ap=[base.ap[0]] + dims)

    with tile.TileContext(nc) as tc:
        with contextlib.ExitStack() as ctx:
            xp = ctx.enter_context(tc.tile_pool(name="xp", bufs=1))
            sp = ctx.enter_context(tc.tile_pool(name="sp", bufs=1))
            php = ctx.enter_context(tc.tile_pool(name="php", bufs=php_bufs))

            xf = xp.tile([PB, 7, XLEN], bf16)
            acc = sp.tile([PB, NQ], f32)
            cpi2 = sp.tile([PB, 1], f32)
            nc.vector.memset(cpi2[:], PI / 2)

            fd_eng = nc.gpsimd if pool_folds else nc.vector

            for rep in range(reps):
                st1 = contextlib.ExitStack()
                p_k = st1.enter_context(tc.tile_pool(name="p_k", bufs=1))
                p_phr = st1.enter_context(tc.tile_pool(name="p_phr",
                                                       bufs=phr_bufs))
                p_sig = st1.enter_context(tc.tile_pool(name="p_sig", bufs=1))
                p_t = st1.enter_context(tc.tile_pool(name="p_t", bufs=2))
                p_u = st1.enter_context(tc.tile_pool(name="p_u", bufs=2))
                p_cb = st1.enter_context(tc.tile_pool(name="p_cb", bufs=1))

                # CB row0 = ones so x0..x5 fold into one multiply
                cb = p_cb.tile([PB, 7, FC], bf16, tag="cb")
                nc.vector.memset(cb[:, 0, :], 1.0)

                for ch in range(CN):
                    cs = ch * FC
                    poff = ch * FR * ROWP

                    phic = php.tile([PB, M], f32, tag="phic")
                    nc.sync.dma_start(phic[:], pd_flat[:, cs * NA:(cs + FC) * NA])
                    if mode == "dma":
                        nc.vector.tensor_reduce(acc[:, 0:1], phic[:, 0:8],
                                                axis=X, op=Op.add)
                        continue

                    # phir = phi wrapped to (-pi, pi], written as bf16
                    # ANGLE-MAJOR ROWS so every downstream op is contiguous
                    js = [[1, NA], [NA, FC]]      # (angle, site) on interleaved
                    phr = p_phr.tile([PB, NA, FC], bf16, tag="phr")
                    if wrap_arw:
                        # two single-period wraps cover |phi| < 5pi
                        w1 = p_k.tile([PB, M], f32, tag="k")
                        nc.vector.add_range_wrap(w1[:], phic[:], 0.0, PI, 2 * PI)
                        nc.vector.add_range_wrap(
                            ap(phr, 0, [[FC, NA], [1, FC]]),
                            ap(w1, 0, js), 0.0, PI, 2 * PI)
                    else:
                        # k = RNE(phi/2pi); phir = phi - 2pi*k
                        k = p_k.tile([PB, M], i16, tag="k")
                        nc.vector.tensor_scalar(k[:], phic[:], 1.0 / (2 * PI),
                                                None, op0=Op.mult)
                        nc.vector.scalar_tensor_tensor(
                            ap(phr, 0, [[FC, NA], [1, FC]]),
                            ap(k, 0, js), -2 * PI, ap(phic, 0, js),
                            op0=Op.mult, op1=Op.add)

                    tt = p_t.tile([PB, NA, FC], bf16, tag="t")
                    nc.scalar.activation(tt[:], phr[:], Act.Sin)
                    uu = p_u.tile([PB, NA, FC], bf16, tag="u")
                    if abs_cos:
                        # sigma rows 0..4 only; u = cos(phir) via
                        # Sin(pi/2 - |phir|) (cos is even), c5 free
                        sig = p_sig.tile([PB, 5, FC], bf16, tag="sig")
                        nc.scalar.activation(sig[:], phr[:, 0:5, :], Act.Sign)
                        nc.scalar.activation(phr[:], phr[:], Act.Abs)
                        nc.scalar.activation(uu[:], phr[:], Act.Sin,
                                             bias=cpi2[:], scale=-1.0)
                        # s~_j = sigma*t ; c~_j = sigma*u (j<5, in place)
                        nc.vector.tensor_tensor(tt[:, 0:5, :], tt[:, 0:5, :],
                                                sig[:], op=Op.mult)
                        nc.vector.tensor_tensor(uu[:, 0:5, :], uu[:, 0:5, :],
                                                sig[:], op=Op.mult)
                    else:
                        sig = p_sig.tile([PB, NA, FC], bf16, tag="sig")
                        nc.scalar.activation(sig[:], phr[:], Act.Sign)
                        # arg2 = sigma*pi/2 - phir (in place); u = Sin(arg2)
                        nc.vector.scalar_tensor_tensor(
                            phr[:], sig[:], PI / 2, phr[:],
                            op0=Op.mult, op1=Op.subtract)
                        nc.scalar.activation(uu[:], phr[:], Act.Sin)
                        # s~_j = sigma*t (j<5); c5 = sigma5*u5
                        nc.vector.tensor_tensor(tt[:, 0:5, :], tt[:, 0:5, :],
                                                sig[:, 0:5, :], op=Op.mult)
                        nc.vector.tensor_tensor(uu[:, 5, :], uu[:, 5, :],
                                                sig[:, 5, :], op=Op.mult)

                    # cumprod tree: CB rows [1,C1,C2,C3,C4,C5,Q45]
                    if cp_act:
                        nc.scalar.copy(cb[:, 1, :], tt[:, 0, :])
                    else:
                        nc.vector.tensor_copy(cb[:, 1, :], tt[:, 0, :])
                    # P: rows {2,4,6} = t{0,2,4} * t{1,3,5}
                    nc.vector.tensor_tensor(
                        ap(cb, 2 * FC, [[2 * FC, 3], [1, FC]]),
                        ap(tt, 0, [[2 * FC, 3], [1, FC]]),
                        ap(tt, FC, [[2 * FC, 3], [1, FC]]), op=Op.mult)
                    # C4: row4 *= row2
                    nc.vector.tensor_tensor(cb[:, 4, :], cb[:, 4, :],
                                            cb[:, 2, :], op=Op.mult)
                    # C3,C5: rows {3,5} = rows {2,4} * t{2,4}
                    nc.vector.tensor_tensor(
                        ap(cb, 3 * FC, [[2 * FC, 2], [1, FC]]),
                        ap(cb, 2 * FC, [[2 * FC, 2], [1, FC]]),
                        ap(tt, 2 * FC, [[2 * FC, 2], [1, FC]]), op=Op.mult)

                    def xrow(kk_, n=1):
                        return ap(xf, kk_ * XLEN + poff,
                                  [[XLEN, n], [ROWP, FR], [1, L]])

                    rowsFR = [[L, FR], [1, L]]
                    # x0..x5 = u rows0..5 * CB rows0..5 ; x6 = C4 * Q45
                    nc.vector.tensor_tensor(
                        xrow(0, 6),
                        ap(uu, 0, [[FC, 6]] + rowsFR),
                        ap(cb, 0, [[FC, 6]] + rowsFR), op=Op.mult)
                    nc.vector.tensor_tensor(
                        xrow(6), ap(cb, 4 * FC, rowsFR),
                        ap(cb, 6 * FC, rowsFR), op=Op.mult)

                if mode == "dma":
                    st1.close()
                    continue

                # pads: col 64 = col 0 per row; row 64 = row 0
                pad_cp = nc.scalar.copy if cp_act else nc.vector.tensor_copy
                pad_cp(ap(xf, L, [[XLEN, 7], [ROWP, NROW]]),
                       ap(xf, 0, [[XLEN, 7], [ROWP, NROW]]))
                pad_cp(ap(xf, NROW * ROWP, [[XLEN, 7], [1, ROWP]]),
                       ap(xf, 0, [[XLEN, 7], [1, ROWP]]))

                st1.close()
                if mode == "stage1":
                    nc.vector.tensor_reduce(acc[:, 0:1], xf[:, 0, 0:8],
                                            axis=X, op=Op.add)
                    continue

                # ======== stage 2: quarters, both dirs side by side ========
                st2 = contextlib.ExitStack()
                p_mt = st2.enter_context(tc.tile_pool(name="p_mt",
                                                      bufs=mt_bufs))

                for q in range(NQ):
                    mt = p_mt.tile([PB, 13, DS], bf16, tag="mt")
                    o = q * NRQ * ROWP

                    def xv(k0, n, extra=0):
                        return ap(xf, k0 * XLEN + o + extra,
                                  [[XLEN, n], [ROWP, NRQ], [1, L]])

                    def mv(r0, n, d):
                        return ap(mt, r0 * DS + d * QS,
                                  [[DS, n], [L, NRQ], [1, L]])

                    for d, off in ((0, ROWP), (1, 1)):
                        # m_k rows 0-6; pq rows 7-9 (zr*zi'), 10-12 (zi*zr')
                        nc.vector.tensor_tensor(mv(0, 7, d), xv(0, 7),
                                                xv(0, 7, off), op=Op.mult)
                        nc.vector.tensor_tensor(mv(7, 3, d), xv(0, 3),
                                                xv(4, 3, off), op=Op.mult)
                        nc.vector.tensor_tensor(mv(10, 3, d), xv(4, 3),
                                                xv(0, 3, off), op=Op.mult)

                    def mf(r0, n, stride=None):
                        return ap(mt, r0 * DS, [[stride or DS, n], [1, DS]])

                    # dre: rows0:3 -= rows4:7; rows{0,1} += rows{2,3}
                    nc.vector.tensor_tensor(mf(0, 3), mf(0, 3), mf(4, 3),
                                            op=Op.subtract)
                    nc.vector.tensor_tensor(mf(0, 2), mf(0, 2), mf(2, 2),
                                            op=Op.add)
                    # dim partial: rows7:10 += rows10:13
                    fd_eng.tensor_tensor(mf(7, 3), mf(7, 3), mf(10, 3),
                                         op=Op.add)
                    # fused: row0 += row1 AND row7 += row8
                    nc.vector.tensor_tensor(mf(0, 2, 7 * DS), mf(0, 2, 7 * DS),
                                            mf(1, 2, 7 * DS), op=Op.add)
                    # dim: row1 = row7 + row9
                    fd_eng.tensor_tensor(mf(1, 1), mf(7, 1), mf(9, 1),
                                         op=Op.add)
                    # Square+accum over rows {0,1} (dre, dim; both dirs)
                    nc.scalar.activation(mf(0, 2), mf(0, 2), Act.Square,
                                         accum_out=acc[:, q:q + 1])
                st2.close()

            # ======== final reduce + affine ========
            acc_src = acc[:] if mode == "full" else acc[:, 0:1]
            stot = sp.tile([PB, 1], f32)
            nc.vector.tensor_reduce(stot[:], acc_src, axis=X, op=Op.add)
            res = sp.tile([PB, 1], f32)
            nc.vector.tensor_scalar(res[:], stot[:], -NBETA, NBETA * 2.0 * S,
                                    op0=Op.mult, op1=Op.add)
            nc.sync.dma_start(out_d[:], res[:])

    nc.finalize()
    return nc


def kernel(phi, shift):
    from concourse.bass_utils import run_bass_kernel_spmd

    phi = np.ascontiguousarray(np.asarray(phi, dtype=np.float32))
    shift = np.asarray(shift, dtype=np.int32)
    key = (shift.tobytes(), 1)
    if key not in _cache:
        _cache[key] = _build(shift)
    nc = _cache[key]

    in_maps = [{"phi": phi[i * PB:(i + 1) * PB]} for i in range(NCORES)]
    res = run_bass_kernel_spmd(nc, in_maps, core_ids=list(range(NCORES)))
    out = np.concatenate([r["out"] for r in res.results], axis=0)
    return out.astype(np.float32)



# revision 29
# speedup vs baseline: 1.0596x; 1.0596x over previous
"""CP(n) lattice action kernel for Trainium2 (8 NeuronCores, Bass/Tile).

Fast path for the roll-structured shift (nearest-neighbor on the 64x64
lattice); general gather-based fallback for arbitrary shift tables.
See _build_fast for the layout/math notes.
"""
import contextlib
import sys

import numpy as np

sys.path.insert(0, "/opt/trn_rl_repo")

B, S, NA = 1024, 4096, 6
NCORES = 8
PB = B // NCORES          # 128 batches per core
L = 64                    # lattice row length
NROW = S // L             # 64 rows
ROWP = L + 1              # padded row length
XLEN = NROW * ROWP + ROWP  # 4225: 64 padded rows + wrap row
PI = float(np.pi)
NBETA = 4.0               # N * BETA


_cache = {}


def _detect_roll(shift):
    idx = np.arange(S).reshape(L, L)
    s0 = np.roll(idx, -1, axis=0).ravel()
    s1 = np.roll(idx, -1, axis=1).ravel()
    return np.array_equal(shift[0], s0) and np.array_equal(shift[1], s1)


def _runs(perm):
    runs = []
    st = 0
    for i in range(1, len(perm) + 1):
        if i == len(perm) or perm[i] != perm[i - 1] + 1:
            runs.append((st, int(perm[st]), i - st))
            st = i
    return runs


FAST_VER = 3


def _build(shift, reps=1, mode="full", dummy_io=False):
    if _detect_roll(shift):
        if FAST_VER >= 3 and mode == "full":
            return _build_fast3(reps=reps, dummy_io=dummy_io)
        if FAST_VER >= 2:
            return _build_fast2(reps=reps, mode=mode, dummy_io=dummy_io)
        return _build_fast(reps=reps, mode=mode, dummy_io=dummy_io)
    return _build_general(shift, reps=reps, mode=mode)


GC1 = 2048
GN1 = S // GC1
GC2 = 2048
GN2 = S // GC2


def _build_general(shift, reps=1, mode="full"):
    import concourse.bass as bass
    import concourse.tile as tile
    from concourse import bacc, mybir

    f32 = mybir.dt.float32
    bf16 = mybir.dt.bfloat16
    i16 = mybir.dt.int16
    Act = mybir.ActivationFunctionType
    Op = mybir.AluOpType
    X = mybir.AxisListType.X

    roll = _detect_roll(shift)

    nc = bacc.Bacc(None, target_bir_lowering=False)
    phi_d = nc.dram_tensor("phi", [PB, S, NA], f32, kind="ExternalInput")
    out_d = nc.dram_tensor("out", [PB, 1], f32, kind="ExternalOutput")
    pd_flat = phi_d[:].rearrange("p s a -> p (s a)")

    with tile.TileContext(nc) as tc:
        with contextlib.ExitStack() as ctx:
            xfull_pool = ctx.enter_context(tc.tile_pool(name="xfull", bufs=1))
            small_pool = ctx.enter_context(tc.tile_pool(name="small", bufs=1))

            NACC = GN2
            acc = small_pool.tile([PB, NACC], f32)
            if roll:
                xf = xfull_pool.tile([PB, 7, S], bf16)
                xg = None
            else:
                # site-major cells [site, 8] so gpsimd.ap_gather can fetch
                # whole 7-component cells per shift index
                xf = None
                xg = xfull_pool.tile([PB, S, 8], bf16)
                idx_sb = []
                for d in range(2):
                    wrapped = np.zeros((PB, S // 16), np.int16)
                    base = shift[d].reshape(S // 16, 16).T.astype(np.int16)
                    for g in range(PB // 16):
                        wrapped[16 * g:16 * (g + 1)] = base
                    hdl = nc.inline_tensor(wrapped, name=f"shift_idx_{d}")
                    t_ = small_pool.tile([PB, S // 16], mybir.dt.int16, tag=f"idx{d}")
                    nc.sync.dma_start(t_[:], hdl[:])
                    idx_sb.append(t_)

            for rep in range(reps):
                # ======== stage 1: wrap + trig + embedding ========
                st1 = contextlib.ExitStack()
                p_phi = st1.enter_context(tc.tile_pool(name="p_phi", bufs=1))
                p_ks = st1.enter_context(tc.tile_pool(name="p_ks", bufs=1))
                p_t = st1.enter_context(tc.tile_pool(name="p_t", bufs=1))
                p_u = st1.enter_context(tc.tile_pool(name="p_u", bufs=1))
                p_cum = st1.enter_context(tc.tile_pool(name="p_cum", bufs=1))

                for ch in range(GN1):
                    cs = ch * GC1
                    M = GC1 * NA

                    phic = p_phi.tile([PB, M], f32, tag="phic")
                    nc.sync.dma_start(phic[:], pd_flat[:, cs * NA:(cs + GC1) * NA])

                    if mode == "dma":
                        nc.vector.tensor_reduce(acc[:, 0:1], phic[:, 0:8],
                                                axis=X, op=Op.add)
                        continue

                    # k = round(phi/2pi) as int16
                    k = p_ks.tile([PB, M], i16, tag="ks")
                    nc.vector.tensor_scalar(k[:], phic[:], 1.0 / (2 * PI), None,
                                            op0=Op.mult)
                    # phir = (k * -2pi) + phi   (in place)
                    nc.vector.scalar_tensor_tensor(
                        phic[:], k[:], -2 * PI, phic[:], op0=Op.mult, op1=Op.add)

                    # sigma, t = Sin(phir)  (interleaved site-major, bf16)
                    sig = p_ks.tile([PB, M], bf16, tag="ks")
                    nc.scalar.activation(sig[:], phic[:], Act.Sign)
                    tt = p_t.tile([PB, M], bf16, tag="t")
                    nc.scalar.activation(tt[:], phic[:], Act.Sin)
                    # arg2 = sigma*pi/2 - phir (in place over phir)
                    nc.vector.scalar_tensor_tensor(
                        phic[:], sig[:], PI / 2, phic[:],
                        op0=Op.mult, op1=Op.subtract)
                    # u = Sin(arg2) = sigma*cos(phir)
                    uu = p_u.tile([PB, M], bf16, tag="u")
                    nc.scalar.activation(uu[:], phic[:], Act.Sin)

                    def ang(tile_, j, n=1):
                        ap = tile_[:]
                        if n == 1:
                            return bass.AP(tensor=ap.tensor, offset=ap.offset + j,
                                           ap=[ap.ap[0], [NA, GC1]])
                        return bass.AP(tensor=ap.tensor, offset=ap.offset + j,
                                       ap=[ap.ap[0], [NA, GC1], [1, n]])

                    # s_j = sigma*t for j<5 (in place on t)
                    nc.vector.tensor_tensor(ang(tt, 0, 5), ang(tt, 0, 5),
                                            ang(sig, 0, 5), op=Op.mult)
                    # c_5 = sigma*u at j=5 (in place on u)
                    nc.vector.tensor_tensor(ang(uu, 5), ang(uu, 5),
                                            ang(sig, 5), op=Op.mult)

                    # cumprod + x build into xf rows / xg cells
                    cumA = p_cum.tile([PB, GC1], bf16, tag="cumA")
                    cumB = p_cum.tile([PB, GC1], bf16, tag="cumB")
                    if roll:
                        xs = xf[:, :, cs:cs + GC1]
                        xk = [xs[:, k, :] for k in range(7)]
                    else:
                        gap = xg[:]
                        xk = [bass.AP(tensor=gap.tensor,
                                      offset=gap.offset + cs * 8 + k,
                                      ap=[gap.ap[0], [8, GC1]])
                              for k in range(7)]
                    nc.vector.tensor_copy(xk[0], ang(uu, 0))
                    nc.vector.tensor_tensor(xk[1], ang(uu, 1), ang(tt, 0),
                                            op=Op.mult)
                    nc.vector.tensor_tensor(cumA[:], ang(tt, 0), ang(tt, 1),
                                            op=Op.mult)
                    nc.vector.tensor_tensor(xk[2], ang(uu, 2), cumA[:],
                                            op=Op.mult)
                    nc.vector.tensor_tensor(cumB[:], cumA[:], ang(tt, 2),
                                            op=Op.mult)
                    nc.vector.tensor_tensor(xk[3], ang(uu, 3), cumB[:],
                                            op=Op.mult)
                    nc.vector.tensor_tensor(cumA[:], cumB[:], ang(tt, 3),
                                            op=Op.mult)
                    nc.vector.tensor_tensor(xk[4], ang(uu, 4), cumA[:],
                                            op=Op.mult)
                    nc.vector.tensor_tensor(cumB[:], cumA[:], ang(tt, 4),
                                            op=Op.mult)
                    nc.vector.tensor_tensor(xk[5], ang(uu, 5), cumB[:],
                                            op=Op.mult)
                    nc.vector.tensor_tensor(xk[6], cumB[:], ang(tt, 5),
                                            op=Op.mult)

                st1.close()
                if mode in ("dma", "stage1"):
                    continue

                # ======== stage 2: neighbor products ========
                st2 = contextlib.ExitStack()
                p_xp = st2.enter_context(tc.tile_pool(name="p_xp", bufs=1))
                p_m = st2.enter_context(tc.tile_pool(name="p_m", bufs=1))
                p_pq = st2.enter_context(tc.tile_pool(name="p_pq", bufs=1))

                for ch in range(GN2):
                    cs = ch * GC2
                    if roll:
                        xs = xf[:, :, cs:cs + GC2]
                    else:
                        gap = xg[:]
                        xs = None
                        xg_k = lambda k0, n, off=0: bass.AP(
                            tensor=gap.tensor,
                            offset=gap.offset + cs * 8 + k0,
                            ap=[gap.ap[0], [1, n], [8, GC2]])

                    # double-width: both dirs side by side, shared folds
                    m = p_m.tile([PB, 7, 2 * GC2], bf16, tag="m")
                    pq = p_pq.tile([PB, 6, 2 * GC2], bf16, tag="pq")

                    for d in (0, 1):
                        if roll and d == 0:
                            lo = cs + L
                            if lo + GC2 <= S:
                                xp_ap = xf[:, :, lo:lo + GC2]
                            else:
                                xp = p_xp.tile([PB, 7, GC2], bf16, tag="xp")
                                mn = S - lo
                                nc.vector.tensor_copy(xp[:, :, 0:mn],
                                                      xf[:, :, lo:S])
                                nc.vector.tensor_copy(xp[:, :, mn:GC2],
                                                      xf[:, :, 0:GC2 - mn])
                                xp_ap = xp[:]
                        elif roll and d == 1:
                            xp = p_xp.tile([PB, 7, GC2], bf16, tag="xp")
                            nrow = GC2 // L
                            src = bass.AP(
                                tensor=xf.tensor, offset=xf[:].offset + cs + 1,
                                ap=[xf[:].ap[0], [S, 7], [L, nrow], [1, L - 1]])
                            dst = bass.AP(
                                tensor=xp.tensor, offset=xp[:].offset,
                                ap=[xp[:].ap[0], [GC2, 7], [L, nrow], [1, L - 1]])
                            nc.gpsimd.tensor_copy(dst, src)
                            srcw = bass.AP(
                                tensor=xf.tensor, offset=xf[:].offset + cs,
                                ap=[xf[:].ap[0], [S, 7], [L, nrow]])
                            dstw = bass.AP(
                                tensor=xp.tensor, offset=xp[:].offset + L - 1,
                                ap=[xp[:].ap[0], [GC2, 7], [L, nrow]])
                            nc.gpsimd.tensor_copy(dstw, srcw)
                            xp_ap = xp[:]
                        else:
                            xpg = p_xp.tile([PB, GC2, 8], bf16, tag="xp")
                            nc.gpsimd.ap_gather(
                                xpg[:], xg[:],
                                idx_sb[d][:, cs // 16:(cs + GC2) // 16],
                                channels=PB, num_elems=S, d=8, num_idxs=GC2)
                            gp = xpg[:]
                            xp_k = lambda k0, n: bass.AP(
                                tensor=gp.tensor, offset=gp.offset + k0,
                                ap=[gp.ap[0], [1, n], [8, GC2]])

                        ms = m[:, :, d * GC2:(d + 1) * GC2]
                        pqs = pq[:, :, d * GC2:(d + 1) * GC2]
                        if roll:
                            nc.vector.tensor_tensor(ms, xs, xp_ap, op=Op.mult)
                            nc.vector.tensor_tensor(
                                pqs[:, 0:3, :], xs[:, 0:3, :],
                                xp_ap[:, 4:7, :], op=Op.mult)
                            nc.vector.tensor_tensor(
                                pqs[:, 3:6, :], xs[:, 4:7, :],
                                xp_ap[:, 0:3, :], op=Op.mult)
                        else:
                            nc.vector.tensor_tensor(ms, xg_k(0, 7),
                                                    xp_k(0, 7), op=Op.mult)
                            nc.vector.tensor_tensor(pqs[:, 0:3, :], xg_k(0, 3),
                                                    xp_k(4, 3), op=Op.mult)
                            nc.vector.tensor_tensor(pqs[:, 3:6, :], xg_k(4, 3),
                                                    xp_k(0, 3), op=Op.mult)

                    # shared folds across both dirs (double width)
                    nc.vector.tensor_tensor(m[:, 0:3, :], m[:, 0:3, :],
                                            m[:, 4:7, :], op=Op.subtract)
                    nc.vector.tensor_tensor(m[:, 0:2, :], m[:, 0:2, :],
                                            m[:, 2:4, :], op=Op.add)
                    nc.vector.tensor_tensor(m[:, 0, :], m[:, 0, :],
                                            m[:, 1, :], op=Op.add)
                    nc.vector.tensor_tensor(pq[:, 0:3, :], pq[:, 0:3, :],
                                            pq[:, 3:6, :], op=Op.add)
                    nc.vector.tensor_tensor(pq[:, 0, :], pq[:, 0, :],
                                            pq[:, 1, :], op=Op.add)
                    nc.vector.tensor_tensor(m[:, 1, :], pq[:, 0, :],
                                            pq[:, 2, :], op=Op.add)

                    # one fused square+accum per chunk (wr,wi x both dirs),
                    # squared in place over the fold results
                    nc.scalar.activation(m[:, 0:2, :], m[:, 0:2, :], Act.Square,
                                         accum_out=acc[:, ch:ch + 1])

                st2.close()

            # ======== final reduce + affine ========
            stot = small_pool.tile([PB, 1], f32)
            nc.vector.tensor_reduce(stot[:], acc[:], axis=X, op=Op.add)
            res = small_pool.tile([PB, 1], f32)
            nc.vector.tensor_scalar(res[:], stot[:], -NBETA, NBETA * 2.0 * S,
                                    op0=Op.mult, op1=Op.add)
            nc.sync.dma_start(out_d[:], res[:])

    nc.finalize()
    return nc



def _build_fast(reps=1, mode="full", dummy_io=False):
    """Roll-shift fast path, instruction-count-minimal.

    This runtime is per-instruction-overhead dominated (~40-100us per
    instruction regardless of size), so the kernel uses FEW, HUGE
    instructions: 2 stage-1 chunks of 2048 sites, and a single full-S
    pass per direction in stage 2 (~65 instructions per rep vs ~75 for
    the previous baseline).

    Layout: x embedding stored angle-major in a 65-padded site layout
    (site (r,c) at p=65r+c; col 64 of each row = col 0 copy; row 64 =
    row 0 copy), so both lattice neighbor shifts are pure offset views
    (+1 for d=1, +65 for d=0) -- no gather/shift copies at all.

    Math (identical to the proven baseline): wrap via k=RNE(phi/2pi),
    phir=phi-2pi*k; sigma=Sign(phir); s_j=sigma*Sin(phir) (j<5),
    c_j=Sin(sigma*pi/2-phir)=sigma*cos(phir) (c_5 fixed up by sigma);
    hyperspherical embedding x via cumprod; per dir:
    m_k=x_k x'_k, pq = zr*zi' / zi*zr'; dre/dim folds; ACT Square+accum.
    """
    import concourse.bass as bass
    import concourse.tile as tile
    from concourse import bacc, mybir

    f32 = mybir.dt.float32
    bf16 = mybir.dt.bfloat16
    i16 = mybir.dt.int16
    Act = mybir.ActivationFunctionType
    Op = mybir.AluOpType
    X = mybir.AxisListType.X

    FC = 2048                 # stage-1 chunk (sites)
    FN = S // FC
    FR = FC // L              # rows per chunk
    M = FC * NA

    nc = bacc.Bacc(None, target_bir_lowering=False)
    phi_kind = "Internal" if dummy_io else "ExternalInput"
    phi_d = nc.dram_tensor("phi", [PB, S, NA], f32, kind=phi_kind)
    out_d = nc.dram_tensor("out", [PB, 1], f32, kind="ExternalOutput")
    pd_flat = phi_d[:].rearrange("p s a -> p (s a)")

    def ap(tile_, off, dims):
        base = tile_[:]
        return bass.AP(tensor=base.tensor, offset=base.offset + off,
                       ap=[base.ap[0]] + dims)

    with tile.TileContext(nc) as tc:
        with contextlib.ExitStack() as ctx:
            xp = ctx.enter_context(tc.tile_pool(name="xp", bufs=1))
            sp = ctx.enter_context(tc.tile_pool(name="sp", bufs=1))

            xf = xp.tile([PB, 7, XLEN], bf16)
            acc = sp.tile([PB, max(reps, 1)], f32)

            for rep in range(reps):
                st1 = contextlib.ExitStack()
                p_phi = st1.enter_context(tc.tile_pool(name="p_phi", bufs=1))
                p_ks = st1.enter_context(tc.tile_pool(name="p_ks", bufs=1))
                p_t = st1.enter_context(tc.tile_pool(name="p_t", bufs=1))
                p_u = st1.enter_context(tc.tile_pool(name="p_u", bufs=1))
                p_cum = st1.enter_context(tc.tile_pool(name="p_cum", bufs=1))

                for ch in range(FN):
                    cs = ch * FC
                    poff = (cs // L) * ROWP

                    phic = p_phi.tile([PB, M], f32, tag="phic")
                    nc.sync.dma_start(phic[:], pd_flat[:, cs * NA:(cs + FC) * NA])
                    if mode == "dma":
                        nc.vector.tensor_reduce(acc[:, 0:1], phic[:, 0:8],
                                                axis=X, op=Op.add)
                        continue

                    # k = RNE(phi/2pi); phir = phi - 2pi*k (in place)
                    k = p_ks.tile([PB, M], i16, tag="ks")
                    nc.vector.tensor_scalar(k[:], phic[:], 1.0 / (2 * PI), None,
                                            op0=Op.mult)
                    nc.vector.scalar_tensor_tensor(
                        phic[:], k[:], -2 * PI, phic[:], op0=Op.mult, op1=Op.add)

                    sig = p_ks.tile([PB, M], bf16, tag="ks")
                    nc.scalar.activation(sig[:], phic[:], Act.Sign)
                    tt_ = p_t.tile([PB, M], bf16, tag="t")
                    nc.scalar.activation(tt_[:], phic[:], Act.Sin)
                    # arg2 = sigma*pi/2 - phir (in place); u = Sin(arg2)
                    nc.vector.scalar_tensor_tensor(
                        phic[:], sig[:], PI / 2, phic[:],
                        op0=Op.mult, op1=Op.subtract)
                    uu = p_u.tile([PB, M], bf16, tag="u")
                    nc.scalar.activation(uu[:], phic[:], Act.Sin)

                    def ang(tile_, j, n=1):
                        a_ = tile_[:]
                        if n == 1:
                            return bass.AP(tensor=a_.tensor,
                                           offset=a_.offset + j,
                                           ap=[a_.ap[0], [NA, FC]])
                        return bass.AP(tensor=a_.tensor, offset=a_.offset + j,
                                       ap=[a_.ap[0], [NA, FC], [1, n]])

                    # s_j = sigma*t (j<5); c_5 = sigma*u (in place)
                    nc.vector.tensor_tensor(ang(tt_, 0, 5), ang(tt_, 0, 5),
                                            ang(sig, 0, 5), op=Op.mult)
                    nc.vector.tensor_tensor(ang(uu, 5), ang(uu, 5),
                                            ang(sig, 5), op=Op.mult)

                    # cumprod + x build into padded xf rows
                    cumA = p_cum.tile([PB, FC], bf16, tag="cumA")
                    cumB = p_cum.tile([PB, FC], bf16, tag="cumB")

                    def xrow(kk):
                        return ap(xf, kk * XLEN + poff, [[ROWP, FR], [1, L]])

                    def angr(tile_, j):
                        a_ = tile_[:]
                        return bass.AP(tensor=a_.tensor, offset=a_.offset + j,
                                       ap=[a_.ap[0], [NA * L, FR], [NA, L]])

                    flat = [[L, FR], [1, L]]
                    TT = nc.vector.tensor_tensor
                    nc.vector.tensor_copy(xrow(0), angr(uu, 0))
                    TT(xrow(1), angr(uu, 1), angr(tt_, 0), op=Op.mult)
                    TT(ap(cumA, 0, flat), angr(tt_, 0), angr(tt_, 1), op=Op.mult)
                    TT(xrow(2), angr(uu, 2), ap(cumA, 0, flat), op=Op.mult)
                    TT(ap(cumB, 0, flat), ap(cumA, 0, flat), angr(tt_, 2),
                       op=Op.mult)
                    TT(xrow(3), angr(uu, 3), ap(cumB, 0, flat), op=Op.mult)
                    TT(ap(cumA, 0, flat), ap(cumB, 0, flat), angr(tt_, 3),
                       op=Op.mult)
                    TT(xrow(4), angr(uu, 4), ap(cumA, 0, flat), op=Op.mult)
                    TT(ap(cumB, 0, flat), ap(cumA, 0, flat), angr(tt_, 4),
                       op=Op.mult)
                    TT(xrow(5), angr(uu, 5), ap(cumB, 0, flat), op=Op.mult)
                    TT(xrow(6), ap(cumB, 0, flat), angr(tt_, 5), op=Op.mult)

                # col pads for all rows at once, then the wrap row
                nc.scalar.copy(
                    ap(xf, L, [[XLEN, 7], [ROWP, NROW]]),
                    ap(xf, 0, [[XLEN, 7], [ROWP, NROW]]))
                nc.scalar.copy(
                    ap(xf, NROW * ROWP, [[XLEN, 7], [1, ROWP]]),
                    ap(xf, 0, [[XLEN, 7], [1, ROWP]]))

                st1.close()
                if mode in ("dma", "stage1"):
                    continue

                # ======== stage 2: one full-S pass per direction ========
                st2 = contextlib.ExitStack()
                p_mt = st2.enter_context(tc.tile_pool(name="p_mt", bufs=1))
                # dir 1 uses rows shifted by +2 so both dirs' dre/dim
                # land in contiguous rows 0-3 and share one Square+accum
                mt = p_mt.tile([PB, 15, S], bf16, tag="mt")

                NR = S // L

                TT = nc.vector.tensor_tensor
                for d, off in ((0, ROWP), (1, 1)):
                    r0 = 2 * d

                    def mrow(i, n=1):
                        return ap(mt, (r0 + i) * S, [[S, n], [L, NR], [1, L]])

                    def mfl(i, n=1):
                        return ap(mt, (r0 + i) * S, [[S, n], [1, S]])

                    def xv(k0, n, o):
                        return ap(xf, k0 * XLEN + o,
                                  [[XLEN, n], [ROWP, NR], [1, L]])

                    # m_k rows 0-6; pq rows 7-9 (zr*zi') and 10-12 (zi*zr')
                    TT(mrow(0, 7), xv(0, 7, 0), xv(0, 7, off), op=Op.mult)
                    TT(mrow(7, 3), xv(0, 3, 0), xv(4, 3, off), op=Op.mult)
                    TT(mrow(10, 3), xv(4, 3, 0), xv(0, 3, off), op=Op.mult)
                    # dre folds: m[0:3]-=m[4:7]; m[0:2]+=m[2:4]
                    TT(mfl(0, 3), mfl(0, 3), mfl(4, 3), op=Op.subtract)
                    TT(mfl(0, 2), mfl(0, 2), mfl(2, 2), op=Op.add)
                    # dim partial: pq[7:10]+=pq[10:13]
                    TT(mfl(7, 3), mfl(7, 3), mfl(10, 3), op=Op.add)
                    # fused: m0+=m1 AND pq7+=pq8 (rows {0,7} += rows {1,8})
                    def mpair(i):
                        return ap(mt, (r0 + i) * S, [[7 * S, 2], [1, S]])
                    TT(mpair(0), mpair(0), mpair(1), op=Op.add)
                    # dim finish: m1 = pq7 + pq9
                    TT(mfl(1), mfl(7), mfl(9), op=Op.add)
                # one Square+accum over rows 0-3 (dre0, dim0, dre1, dim1)
                nc.scalar.activation(
                    ap(mt, 0, [[S, 4], [1, S]]),
                    ap(mt, 0, [[S, 4], [1, S]]),
                    Act.Square, accum_out=acc[:, rep:rep + 1])
                st2.close()

            # ======== final reduce + affine ========
            stot = sp.tile([PB, 1], f32)
            nc.vector.tensor_reduce(stot[:], acc[:, reps - 1:reps],
                                    axis=X, op=Op.add)
            res = sp.tile([PB, 1], f32)
            nc.vector.tensor_scalar(res[:], stot[:], -NBETA, NBETA * 2.0 * S,
                                    op0=Op.mult, op1=Op.add)
            nc.sync.dma_start(out_d[:], res[:])

    nc.finalize()
    return nc


def _build_fast3(reps=1, mode="full", dummy_io=False):
    """Software-pipelined roll-shift fast path (v3).

    Same math and layouts as _build_fast2, but stage-2 quarter q of rep
    i-1 is EMITTED just before stage-1 chunk q of rep i. Engine streams
    execute in order, so this interleaving lets the ACT trig of the next
    rep run underneath the DVE product/fold work of the current one; the
    xf buffer rotates at quarter granularity via region-level WAR deps
    (chunk q == quarter q), with the wrap row copied late so stage-2's
    torus reads never block the refill. Single-buffered stage-1 tiles +
    one mt buffer keep the concurrent working set under the SBUF limit.
    """
    import concourse.bass as bass
    import concourse.tile as tile
    from concourse import bacc, mybir

    f32 = mybir.dt.float32
    bf16 = mybir.dt.bfloat16
    Act = mybir.ActivationFunctionType
    Op = mybir.AluOpType
    X = mybir.AxisListType.X

    FC = 1024                 # chunk == quarter (sites)
    CN = S // FC
    FR = FC // L
    M = FC * NA
    DS = 2 * FC               # both dirs side by side

    nc = bacc.Bacc(None, target_bir_lowering=False)
    phi_kind = "Internal" if dummy_io else "ExternalInput"
    phi_d = nc.dram_tensor("phi", [PB, S, NA], f32, kind=phi_kind)
    out_d = nc.dram_tensor("out", [PB, 1], f32, kind="ExternalOutput")
    pd_flat = phi_d[:].rearrange("p s a -> p (s a)")

    def ap(tile_, off, dims):
        base = tile_[:]
        return bass.AP(tensor=base.tensor, offset=base.offset + off,
                       ap=[base.ap[0]] + dims)

    with tile.TileContext(nc) as tc:
        with contextlib.ExitStack() as ctx:
            xp = ctx.enter_context(tc.tile_pool(name="xp", bufs=1))
            sp = ctx.enter_context(tc.tile_pool(name="sp", bufs=1))
            php = ctx.enter_context(tc.tile_pool(name="php", bufs=1))
            p_phr = ctx.enter_context(tc.tile_pool(name="p_phr", bufs=1))
            p_sig = ctx.enter_context(tc.tile_pool(name="p_sig", bufs=1))
            p_t = ctx.enter_context(tc.tile_pool(name="p_t", bufs=1))
            p_u = ctx.enter_context(tc.tile_pool(name="p_u", bufs=1))
            p_cb = ctx.enter_context(tc.tile_pool(name="p_cb", bufs=1))
            p_mt = ctx.enter_context(tc.tile_pool(name="p_mt", bufs=1))

            xf = xp.tile([PB, 7, XLEN], bf16)
            acc = sp.tile([PB, CN], f32)
            cpi2 = sp.tile([PB, 1], f32)
            nc.vector.memset(cpi2[:], PI / 2)
            cb = p_cb.tile([PB, 7, FC], bf16, tag="cb")
            nc.vector.memset(cb[:, 0, :], 1.0)
            mt = p_mt.tile([PB, 13, DS], bf16, tag="mt")

            def dma(ch):
                phic = php.tile([PB, M], f32, tag="phic")
                cs = ch * FC
                nc.sync.dma_start(phic[:], pd_flat[:, cs * NA:(cs + FC) * NA])
                return phic

            def wrap(phic):
                # wrap to (-pi, pi] in place, then deinterleave to bf16 rows
                js = [[1, NA], [NA, FC]]
                nc.vector.add_range_wrap(phic[:], phic[:], 0.0, PI, 2 * PI)
                phr = p_phr.tile([PB, NA, FC], bf16, tag="phr")
                nc.vector.add_range_wrap(ap(phr, 0, [[FC, NA], [1, FC]]),
                                         ap(phic, 0, js), 0.0, PI, 2 * PI)
                return phr

            def trig(phr):
                tt = p_t.tile([PB, NA, FC], bf16, tag="t")
                nc.scalar.activation(tt[:], phr[:], Act.Sin)
                sig = p_sig.tile([PB, 5, FC], bf16, tag="sig")
                nc.scalar.activation(sig[:], phr[:, 0:5, :], Act.Sign)
                nc.scalar.activation(phr[:], phr[:], Act.Abs)
                uu = p_u.tile([PB, NA, FC], bf16, tag="u")
                nc.scalar.activation(uu[:], phr[:], Act.Sin,
                                     bias=cpi2[:], scale=-1.0)
                return tt, sig, uu

            def xbuild(ch, tt, sig, uu):
                poff = ch * FR * ROWP
                nc.vector.tensor_tensor(tt[:, 0:5, :], tt[:, 0:5, :],
                                        sig[:], op=Op.mult)
                nc.vector.tensor_tensor(uu[:, 0:5, :], uu[:, 0:5, :],
                                        sig[:], op=Op.mult)
                # cumprod tree: CB rows [1,C1,C2,C3,C4,C5,Q45]
                nc.scalar.copy(cb[:, 1, :], tt[:, 0, :])
                nc.vector.tensor_tensor(
                    ap(cb, 2 * FC, [[2 * FC, 3], [1, FC]]),
                    ap(tt, 0, [[2 * FC, 3], [1, FC]]),
                    ap(tt, FC, [[2 * FC, 3], [1, FC]]), op=Op.mult)
                nc.vector.tensor_tensor(cb[:, 4, :], cb[:, 4, :],
                                        cb[:, 2, :], op=Op.mult)
                nc.vector.tensor_tensor(
                    ap(cb, 3 * FC, [[2 * FC, 2], [1, FC]]),
                    ap(cb, 2 * FC, [[2 * FC, 2], [1, FC]]),
                    ap(tt, 2 * FC, [[2 * FC, 2], [1, FC]]), op=Op.mult)

                def xrow(kk_, n=1):
                    return ap(xf, kk_ * XLEN + poff,
                              [[XLEN, n], [ROWP, FR], [1, L]])

                rowsFR = [[L, FR], [1, L]]
                nc.vector.tensor_tensor(
                    xrow(0, 6),
                    ap(uu, 0, [[FC, 6]] + rowsFR),
                    ap(cb, 0, [[FC, 6]] + rowsFR), op=Op.mult)
                nc.vector.tensor_tensor(
                    xrow(6), ap(cb, 4 * FC, rowsFR),
                    ap(cb, 6 * FC, rowsFR), op=Op.mult)
                # per-chunk col pad: col 64 = col 0 for this chunk's rows
                nc.scalar.copy(
                    ap(xf, poff + L, [[XLEN, 7], [ROWP, FR]]),
                    ap(xf, poff, [[XLEN, 7], [ROWP, FR]]))

            def wraprow():
                nc.scalar.copy(ap(xf, NROW * ROWP, [[XLEN, 7], [1, ROWP]]),
                               ap(xf, 0, [[XLEN, 7], [1, ROWP]]))

            def prod_folds(q):
                o = q * FR * ROWP

                def xv(k0, n, extra=0):
                    return ap(xf, k0 * XLEN + o + extra,
                              [[XLEN, n], [ROWP, FR], [1, L]])

                def mv(r0, n, d):
                    return ap(mt, r0 * DS + d * FC,
                              [[DS, n], [L, FR], [1, L]])

                for d, off in ((0, ROWP), (1, 1)):
                    nc.vector.tensor_tensor(mv(0, 7, d), xv(0, 7),
                                            xv(0, 7, off), op=Op.mult)
                    nc.vector.tensor_tensor(mv(7, 3, d), xv(0, 3),
                                            xv(4, 3, off), op=Op.mult)
                    nc.vector.tensor_tensor(mv(10, 3, d), xv(4, 3),
                                            xv(0, 3, off), op=Op.mult)

                def mf(r0, n, stride=None):
                    return ap(mt, r0 * DS, [[stride or DS, n], [1, DS]])

                nc.vector.tensor_tensor(mf(0, 3), mf(0, 3), mf(4, 3),
                                        op=Op.subtract)
                nc.vector.tensor_tensor(mf(0, 2), mf(0, 2), mf(2, 2),
                                        op=Op.add)
                nc.vector.tensor_tensor(mf(7, 3), mf(7, 3), mf(10, 3),
                                        op=Op.add)
                nc.vector.tensor_tensor(mf(0, 2, 7 * DS), mf(0, 2, 7 * DS),
                                        mf(1, 2, 7 * DS), op=Op.add)
                nc.vector.tensor_tensor(mf(1, 1), mf(7, 1), mf(9, 1),
                                        op=Op.add)

            def square(q):
                nc.scalar.activation(ap(mt, 0, [[DS, 2], [1, DS]]),
                                     ap(mt, 0, [[DS, 2], [1, DS]]),
                                     Act.Square, accum_out=acc[:, q:q + 1])

            # ---- software-pipelined emission ----
            # DVE stream per position: wrap(q) | PF(prev q) | xbuild(q);
            # ACT trig(q) runs under PF(prev q); SQ(prev q) after trig.
            phic = dma(0)
            for rep in range(reps):
                for q in range(CN):
                    phr = wrap(phic)
                    if q + 1 < CN or rep + 1 < reps:
                        phic = dma((q + 1) % CN)      # prefetch next chunk
                    tsu = trig(phr)
                    if rep > 0:
                        prod_folds(q)
                    xbuild(q, *tsu)
                    if rep > 0:
                        square(q)
                wraprow()
            for q in range(CN):
                prod_folds(q)
                square(q)

            stot = sp.tile([PB, 1], f32)
            nc.vector.tensor_reduce(stot[:], acc[:], axis=X, op=Op.add)
            res = sp.tile([PB, 1], f32)
            nc.vector.tensor_scalar(res[:], stot[:], -NBETA, NBETA * 2.0 * S,
                                    op0=Op.mult, op1=Op.add)
            nc.sync.dma_start(out_d[:], res[:])

    nc.finalize()
    return nc


def _build_fast2(reps=1, mode="full", dummy_io=False,
                 pool_folds=False, cp_act=True,
                 phr_bufs=2, mt_bufs=2, php_bufs=1, wrap_arw=True,
                 abs_cos=True):
    """Restructured roll-shift fast path (v2).

    vs _build_fast: angle-major (de-interleaved) trig outputs so all DVE
    ops are row-contiguous; pair-tree cumprod (4 ops instead of a 5-long
    serial chain); both lattice directions packed side by side in each
    stage-2 row so every fold/square instruction covers them at once;
    copies/pads and two of the folds moved to the otherwise-idle gpsimd
    engine; phi pool double-buffered so chunk DMAs overlap compute.

    Math identical to _build_fast: wrap via k=RNE(phi/2pi),
    phir=phi-2pi*k; sigma=Sign(phir); s~=sigma*sin(phir) (|sin|, j<5);
    u=Sin(sigma*pi/2-phir)=sigma*cos(phir) (c5 fixed by extra sigma5);
    hyperspherical x via cumprod tree; per dir m_k/pq products; fold
    tree; Square+accum.
    """
    import concourse.bass as bass
    import concourse.tile as tile
    from concourse import bacc, mybir

    f32 = mybir.dt.float32
    bf16 = mybir.dt.bfloat16
    i16 = mybir.dt.int16
    Act = mybir.ActivationFunctionType
    Op = mybir.AluOpType
    X = mybir.AxisListType.X

    FC = 1024                 # stage-1 chunk (sites)
    CN = S // FC              # 4 chunks
    FR = FC // L              # 16 lattice rows per chunk
    M = FC * NA               # 6144 angles per chunk
    QS = 1024                 # stage-2 quarter (sites per dir)
    NQ = S // QS
    NRQ = QS // L             # 16 lattice rows per quarter
    DS = 2 * QS               # both dirs side by side

    nc = bacc.Bacc(None, target_bir_lowering=False)
    phi_kind = "Internal" if dummy_io else "ExternalInput"
    phi_d = nc.dram_tensor("phi", [PB, S, NA], f32, kind=phi_kind)
    out_d = nc.dram_tensor("out", [PB, 1], f32, kind="ExternalOutput")
    pd_flat = phi_d[:].rearrange("p s a -> p (s a)")

    def ap(tile_, off, dims):
        base = tile_[:]
        return bass.AP(tensor=base.tensor, offset=base.offset + off,
                       ap=[base.ap[0]] + dims)

    with tile.TileContext(nc) as tc:
        with contextlib.ExitStack() as ctx:
            xp = ctx.enter_context(tc.tile_pool(name="xp", bufs=1))
            sp = ctx.enter_context(tc.tile_pool(name="sp", bufs=1))
            php = ctx.enter_context(tc.tile_pool(name="php", bufs=php_bufs))

            xf = xp.tile([PB, 7, XLEN], bf16)
            acc = sp.tile([PB, NQ], f32)
            cpi2 = sp.tile([PB, 1], f32)
            nc.vector.memset(cpi2[:], PI / 2)

            fd_eng = nc.gpsimd if pool_folds else nc.vector

            for rep in range(reps):
                st1 = contextlib.ExitStack()
                p_k = st1.enter_context(tc.tile_pool(name="p_k", bufs=1))
                p_phr = st1.enter_context(tc.tile_pool(name="p_phr",
                                                       bufs=phr_bufs))
                p_sig = st1.enter_context(tc.tile_pool(name="p_sig", bufs=1))
                p_t = st1.enter_context(tc.tile_pool(name="p_t", bufs=2))
                p_u = st1.enter_context(tc.tile_pool(name="p_u", bufs=2))
                p_cb = st1.enter_context(tc.tile_pool(name="p_cb", bufs=1))

                # CB row0 = ones so x0..x5 fold into one multiply
                cb = p_cb.tile([PB, 7, FC], bf16, tag="cb")
                nc.vector.memset(cb[:, 0, :], 1.0)

                for ch in range(CN):
                    cs = ch * FC
                    poff = ch * FR * ROWP

                    phic = php.tile([PB, M], f32, tag="phic")
                    nc.sync.dma_start(phic[:], pd_flat[:, cs * NA:(cs + FC) * NA])
                    if mode == "dma":
                        nc.vector.tensor_reduce(acc[:, 0:1], phic[:, 0:8],
                                                axis=X, op=Op.add)
                        continue

                    # phir = phi wrapped to (-pi, pi], written as bf16
                    # ANGLE-MAJOR ROWS so every downstream op is contiguous
                    js = [[1, NA], [NA, FC]]      # (angle, site) on interleaved
                    phr = p_phr.tile([PB, NA, FC], bf16, tag="phr")
                    if wrap_arw:
                        # two single-period wraps cover |phi| < 5pi
                        w1 = p_k.tile([PB, M], f32, tag="k")
                        nc.vector.add_range_wrap(w1[:], phic[:], 0.0, PI, 2 * PI)
                        nc.vector.add_range_wrap(
                            ap(phr, 0, [[FC, NA], [1, FC]]),
                            ap(w1, 0, js), 0.0, PI, 2 * PI)
                    else:
                        # k = RNE(phi/2pi); phir = phi - 2pi*k
                        k = p_k.tile([PB, M], i16, tag="k")
                        nc.vector.tensor_scalar(k[:], phic[:], 1.0 / (2 * PI),
                                                None, op0=Op.mult)
                        nc.vector.scalar_tensor_tensor(
                            ap(phr, 0, [[FC, NA], [1, FC]]),
                            ap(k, 0, js), -2 * PI, ap(phic, 0, js),
                            op0=Op.mult, op1=Op.add)

                    tt = p_t.tile([PB, NA, FC], bf16, tag="t")
                    nc.scalar.activation(tt[:], phr[:], Act.Sin)
                    uu = p_u.tile([PB, NA, FC], bf16, tag="u")
                    if abs_cos:
                        # sigma rows 0..4 only; u = cos(phir) via
                        # Sin(pi/2 - |phir|) (cos is even), c5 free
                        sig = p_sig.tile([PB, 5, FC], bf16, tag="sig")
                        nc.scalar.activation(sig[:], phr[:, 0:5, :], Act.Sign)
                        nc.scalar.activation(phr[:], phr[:], Act.Abs)
                        nc.scalar.activation(uu[:], phr[:], Act.Sin,
                                             bias=cpi2[:], scale=-1.0)
                        # s~_j = sigma*t ; c~_j = sigma*u (j<5, in place)
                        nc.vector.tensor_tensor(tt[:, 0:5, :], tt[:, 0:5, :],
                                                sig[:], op=Op.mult)
                        nc.vector.tensor_tensor(uu[:, 0:5, :], uu[:, 0:5, :],
                                                sig[:], op=Op.mult)
                    else:
                        sig = p_sig.tile([PB, NA, FC], bf16, tag="sig")
                        nc.scalar.activation(sig[:], phr[:], Act.Sign)
                        # arg2 = sigma*pi/2 - phir (in place); u = Sin(arg2)
                        nc.vector.scalar_tensor_tensor(
                            phr[:], sig[:], PI / 2, phr[:],
                            op0=Op.mult, op1=Op.subtract)
                        nc.scalar.activation(uu[:], phr[:], Act.Sin)
                        # s~_j = sigma*t (j<5); c5 = sigma5*u5
                        nc.vector.tensor_tensor(tt[:, 0:5, :], tt[:, 0:5, :],
                                                sig[:, 0:5, :], op=Op.mult)
                        nc.vector.tensor_tensor(uu[:, 5, :], uu[:, 5, :],
                                                sig[:, 5, :], op=Op.mult)

                    # cumprod tree: CB rows [1,C1,C2,C3,C4,C5,Q45]
                    if cp_act:
                        nc.scalar.copy(cb[:, 1, :], tt[:, 0, :])
                    else:
                        nc.vector.tensor_copy(cb[:, 1, :], tt[:, 0, :])
                    # P: rows {2,4,6} = t{0,2,4} * t{1,3,5}
                    nc.vector.tensor_tensor(
                        ap(cb, 2 * FC, [[2 * FC, 3], [1, FC]]),
                        ap(tt, 0, [[2 * FC, 3], [1, FC]]),
                        ap(tt, FC, [[2 * FC, 3], [1, FC]]), op=Op.mult)
                    # C4: row4 *= row2
                    nc.vector.tensor_tensor(cb[:, 4, :], cb[:, 4, :],
                                            cb[:, 2, :], op=Op.mult)
                    # C3,C5: rows {3,5} = rows {2,4} * t{2,4}
                    nc.vector.tensor_tensor(
                        ap(cb, 3 * FC, [[2 * FC, 2], [1, FC]]),
                        ap(cb, 2 * FC, [[2 * FC, 2], [1, FC]]),
                        ap(tt, 2 * FC, [[2 * FC, 2], [1, FC]]), op=Op.mult)

                    def xrow(kk_, n=1):
                        return ap(xf, kk_ * XLEN + poff,
                                  [[XLEN, n], [ROWP, FR], [1, L]])

                    rowsFR = [[L, FR], [1, L]]
                    # x0..x5 = u rows0..5 * CB rows0..5 ; x6 = C4 * Q45
                    nc.vector.tensor_tensor(
                        xrow(0, 6),
                        ap(uu, 0, [[FC, 6]] + rowsFR),
                        ap(cb, 0, [[FC, 6]] + rowsFR), op=Op.mult)
                    nc.vector.tensor_tensor(
                        xrow(6), ap(cb, 4 * FC, rowsFR),
                        ap(cb, 6 * FC, rowsFR), op=Op.mult)

                if mode == "dma":
                    st1.close()
                    continue

                # pads: col 64 = col 0 per row; row 64 = row 0
                pad_cp = nc.scalar.copy if cp_act else nc.vector.tensor_copy
                pad_cp(ap(xf, L, [[XLEN, 7], [ROWP, NROW]]),
                       ap(xf, 0, [[XLEN, 7], [ROWP, NROW]]))
                pad_cp(ap(xf, NROW * ROWP, [[XLEN, 7], [1, ROWP]]),
                       ap(xf, 0, [[XLEN, 7], [1, ROWP]]))

                st1.close()
                if mode == "stage1":
                    nc.vector.tensor_reduce(acc[:, 0:1], xf[:, 0, 0:8],
                                            axis=X, op=Op.add)
                    continue

                # ======== stage 2: quarters, both dirs side by side ========
                st2 = contextlib.ExitStack()
                p_mt = st2.enter_context(tc.tile_pool(name="p_mt",
                                                      bufs=mt_bufs))

                for q in range(NQ):
                    mt = p_mt.tile([PB, 13, DS], bf16, tag="mt")
                    o = q * NRQ * ROWP

                    def xv(k0, n, extra=0):
                        return ap(xf, k0 * XLEN + o + extra,
                                  [[XLEN, n], [ROWP, NRQ], [1, L]])

                    def mv(r0, n, d):
                        return ap(mt, r0 * DS + d * QS,
                                  [[DS, n], [L, NRQ], [1, L]])

                    for d, off in ((0, ROWP), (1, 1)):
                        # m_k rows 0-6; pq rows 7-9 (zr*zi'), 10-12 (zi*zr')
                        nc.vector.tensor_tensor(mv(0, 7, d), xv(0, 7),
                                                xv(0, 7, off), op=Op.mult)
                        nc.vector.tensor_tensor(mv(7, 3, d), xv(0, 3),
                                                xv(4, 3, off), op=Op.mult)
                        nc.vector.tensor_tensor(mv(10, 3, d), xv(4, 3),
                                                xv(0, 3, off), op=Op.mult)

                    def mf(r0, n, stride=None):
                        return ap(mt, r0 * DS, [[stride or DS, n], [1, DS]])

                    # dre: rows0:3 -= rows4:7; rows{0,1} += rows{2,3}
                    nc.vector.tensor_tensor(mf(0, 3), mf(0, 3), mf(4, 3),
                                            op=Op.subtract)
                    nc.vector.tensor_tensor(mf(0, 2), mf(0, 2), mf(2, 2),
                                            op=Op.add)
                    # dim partial: rows7:10 += rows10:13
                    fd_eng.tensor_tensor(mf(7, 3), mf(7, 3), mf(10, 3),
                                         op=Op.add)
                    # fused: row0 += row1 AND row7 += row8
                    nc.vector.tensor_tensor(mf(0, 2, 7 * DS), mf(0, 2, 7 * DS),
                                            mf(1, 2, 7 * DS), op=Op.add)
                    # dim: row1 = row7 + row9
                    fd_eng.tensor_tensor(mf(1, 1), mf(7, 1), mf(9, 1),
                                         op=Op.add)
                    # Square+accum over rows {0,1} (dre, dim; both dirs)
                    nc.scalar.activation(mf(0, 2), mf(0, 2), Act.Square,
                                         accum_out=acc[:, q:q + 1])
                st2.close()

            # ======== final reduce + affine ========
            acc_src = acc[:] if mode == "full" else acc[:, 0:1]
            stot = sp.tile([PB, 1], f32)
            nc.vector.tensor_reduce(stot[:], acc_src, axis=X, op=Op.add)
            res = sp.tile([PB, 1], f32)
            nc.vector.tensor_scalar(res[:], stot[:], -NBETA, NBETA * 2.0 * S,
                                    op0=Op.mult, op1=Op.add)
            nc.sync.dma_start(out_d[:], res[:])

    nc.finalize()
    return nc


def kernel(phi, shift):
    from concourse.bass_utils import run_bass_kernel_spmd

    phi = np.ascontiguousarray(np.asarray(phi, dtype=np.float32))
    shift = np.asarray(shift, dtype=np.int32)
    key = (shift.tobytes(), 1)
    if key not in _cache:
        _cache[key] = _build(shift)
    nc = _cache[key]

    in_maps = [{"phi": phi[i * PB:(i + 1) * PB]} for i in range(NCORES)]
    res = run_bass_kernel_spmd(nc, in_maps, core_ids=list(range(NCORES)))
    out = np.concatenate([r["out"] for r in res.results], axis=0)
    return out.astype(np.float32)



# revision 31
# speedup vs baseline: 1.1993x; 1.1319x over previous
"""CP(n) lattice action kernel for Trainium2 (8 NeuronCores, Bass/Tile).

Fast path for the roll-structured shift (nearest-neighbor on the 64x64
lattice); general gather-based fallback for arbitrary shift tables.
See _build_fast for the layout/math notes.
"""
import contextlib
import sys

import numpy as np

sys.path.insert(0, "/opt/trn_rl_repo")

B, S, NA = 1024, 4096, 6
NCORES = 8
PB = B // NCORES          # 128 batches per core
L = 64                    # lattice row length
NROW = S // L             # 64 rows
ROWP = L + 1              # padded row length
XLEN = NROW * ROWP + ROWP  # 4225: 64 padded rows + wrap row
PI = float(np.pi)
NBETA = 4.0               # N * BETA


_cache = {}


def _detect_roll(shift):
    idx = np.arange(S).reshape(L, L)
    s0 = np.roll(idx, -1, axis=0).ravel()
    s1 = np.roll(idx, -1, axis=1).ravel()
    return np.array_equal(shift[0], s0) and np.array_equal(shift[1], s1)


def _runs(perm):
    runs = []
    st = 0
    for i in range(1, len(perm) + 1):
        if i == len(perm) or perm[i] != perm[i - 1] + 1:
            runs.append((st, int(perm[st]), i - st))
            st = i
    return runs


FAST_VER = 3


def _build(shift, reps=1, mode="full", dummy_io=False):
    if _detect_roll(shift):
        if FAST_VER >= 3 and mode == "full":
            return _build_fast3(reps=reps, dummy_io=dummy_io)
        if FAST_VER >= 2:
            return _build_fast2(reps=reps, mode=mode, dummy_io=dummy_io)
        return _build_fast(reps=reps, mode=mode, dummy_io=dummy_io)
    return _build_general(shift, reps=reps, mode=mode)


GC1 = 2048
GN1 = S // GC1
GC2 = 2048
GN2 = S // GC2


def _build_general(shift, reps=1, mode="full"):
    import concourse.bass as bass
    import concourse.tile as tile
    from concourse import bacc, mybir

    f32 = mybir.dt.float32
    bf16 = mybir.dt.bfloat16
    i16 = mybir.dt.int16
    Act = mybir.ActivationFunctionType
    Op = mybir.AluOpType
    X = mybir.AxisListType.X

    roll = _detect_roll(shift)

    nc = bacc.Bacc(None, target_bir_lowering=False)
    phi_d = nc.dram_tensor("phi", [PB, S, NA], f32, kind="ExternalInput")
    out_d = nc.dram_tensor("out", [PB, 1], f32, kind="ExternalOutput")
    pd_flat = phi_d[:].rearrange("p s a -> p (s a)")

    with tile.TileContext(nc) as tc:
        with contextlib.ExitStack() as ctx:
            xfull_pool = ctx.enter_context(tc.tile_pool(name="xfull", bufs=1))
            small_pool = ctx.enter_context(tc.tile_pool(name="small", bufs=1))

            NACC = GN2
            acc = small_pool.tile([PB, NACC], f32)
            if roll:
                xf = xfull_pool.tile([PB, 7, S], bf16)
                xg = None
            else:
                # site-major cells [site, 8] so gpsimd.ap_gather can fetch
                # whole 7-component cells per shift index
                xf = None
                xg = xfull_pool.tile([PB, S, 8], bf16)
                idx_sb = []
                for d in range(2):
                    wrapped = np.zeros((PB, S // 16), np.int16)
                    base = shift[d].reshape(S // 16, 16).T.astype(np.int16)
                    for g in range(PB // 16):
                        wrapped[16 * g:16 * (g + 1)] = base
                    hdl = nc.inline_tensor(wrapped, name=f"shift_idx_{d}")
                    t_ = small_pool.tile([PB, S // 16], mybir.dt.int16, tag=f"idx{d}")
                    nc.sync.dma_start(t_[:], hdl[:])
                    idx_sb.append(t_)

            for rep in range(reps):
                # ======== stage 1: wrap + trig + embedding ========
                st1 = contextlib.ExitStack()
                p_phi = st1.enter_context(tc.tile_pool(name="p_phi", bufs=1))
                p_ks = st1.enter_context(tc.tile_pool(name="p_ks", bufs=1))
                p_t = st1.enter_context(tc.tile_pool(name="p_t", bufs=1))
                p_u = st1.enter_context(tc.tile_pool(name="p_u", bufs=1))
                p_cum = st1.enter_context(tc.tile_pool(name="p_cum", bufs=1))

                for ch in range(GN1):
                    cs = ch * GC1
                    M = GC1 * NA

                    phic = p_phi.tile([PB, M], f32, tag="phic")
                    nc.sync.dma_start(phic[:], pd_flat[:, cs * NA:(cs + GC1) * NA])

                    if mode == "dma":
                        nc.vector.tensor_reduce(acc[:, 0:1], phic[:, 0:8],
                                                axis=X, op=Op.add)
                        continue

                    # k = round(phi/2pi) as int16
                    k = p_ks.tile([PB, M], i16, tag="ks")
                    nc.vector.tensor_scalar(k[:], phic[:], 1.0 / (2 * PI), None,
                                            op0=Op.mult)
                    # phir = (k * -2pi) + phi   (in place)
                    nc.vector.scalar_tensor_tensor(
                        phic[:], k[:], -2 * PI, phic[:], op0=Op.mult, op1=Op.add)

                    # sigma, t = Sin(phir)  (interleaved site-major, bf16)
                    sig = p_ks.tile([PB, M], bf16, tag="ks")
                    nc.scalar.activation(sig[:], phic[:], Act.Sign)
                    tt = p_t.tile([PB, M], bf16, tag="t")
                    nc.scalar.activation(tt[:], phic[:], Act.Sin)
                    # arg2 = sigma*pi/2 - phir (in place over phir)
                    nc.vector.scalar_tensor_tensor(
                        phic[:], sig[:], PI / 2, phic[:],
                        op0=Op.mult, op1=Op.subtract)
                    # u = Sin(arg2) = sigma*cos(phir)
                    uu = p_u.tile([PB, M], bf16, tag="u")
                    nc.scalar.activation(uu[:], phic[:], Act.Sin)

                    def ang(tile_, j, n=1):
                        ap = tile_[:]
                        if n == 1:
                            return bass.AP(tensor=ap.tensor, offset=ap.offset + j,
                                           ap=[ap.ap[0], [NA, GC1]])
                        return bass.AP(tensor=ap.tensor, offset=ap.offset + j,
                                       ap=[ap.ap[0], [NA, GC1], [1, n]])

                    # s_j = sigma*t for j<5 (in place on t)
                    nc.vector.tensor_tensor(ang(tt, 0, 5), ang(tt, 0, 5),
                                            ang(sig, 0, 5), op=Op.mult)
                    # c_5 = sigma*u at j=5 (in place on u)
                    nc.vector.tensor_tensor(ang(uu, 5), ang(uu, 5),
                                            ang(sig, 5), op=Op.mult)

                    # cumprod + x build into xf rows / xg cells
                    cumA = p_cum.tile([PB, GC1], bf16, tag="cumA")
                    cumB = p_cum.tile([PB, GC1], bf16, tag="cumB")
                    if roll:
                        xs = xf[:, :, cs:cs + GC1]
                        xk = [xs[:, k, :] for k in range(7)]
                    else:
                        gap = xg[:]
                        xk = [bass.AP(tensor=gap.tensor,
                                      offset=gap.offset + cs * 8 + k,
                                      ap=[gap.ap[0], [8, GC1]])
                              for k in range(7)]
                    nc.vector.tensor_copy(xk[0], ang(uu, 0))
                    nc.vector.tensor_tensor(xk[1], ang(uu, 1), ang(tt, 0),
                                            op=Op.mult)
                    nc.vector.tensor_tensor(cumA[:], ang(tt, 0), ang(tt, 1),
                                            op=Op.mult)
                    nc.vector.tensor_tensor(xk[2], ang(uu, 2), cumA[:],
                                            op=Op.mult)
                    nc.vector.tensor_tensor(cumB[:], cumA[:], ang(tt, 2),
                                            op=Op.mult)
                    nc.vector.tensor_tensor(xk[3], ang(uu, 3), cumB[:],
                                            op=Op.mult)
                    nc.vector.tensor_tensor(cumA[:], cumB[:], ang(tt, 3),
                                            op=Op.mult)
                    nc.vector.tensor_tensor(xk[4], ang(uu, 4), cumA[:],
                                            op=Op.mult)
                    nc.vector.tensor_tensor(cumB[:], cumA[:], ang(tt, 4),
                                            op=Op.mult)
                    nc.vector.tensor_tensor(xk[5], ang(uu, 5), cumB[:],
                                            op=Op.mult)
                    nc.vector.tensor_tensor(xk[6], cumB[:], ang(tt, 5),
                                            op=Op.mult)

                st1.close()
                if mode in ("dma", "stage1"):
                    continue

                # ======== stage 2: neighbor products ========
                st2 = contextlib.ExitStack()
                p_xp = st2.enter_context(tc.tile_pool(name="p_xp", bufs=1))
                p_m = st2.enter_context(tc.tile_pool(name="p_m", bufs=1))
                p_pq = st2.enter_context(tc.tile_pool(name="p_pq", bufs=1))

                for ch in range(GN2):
                    cs = ch * GC2
                    if roll:
                        xs = xf[:, :, cs:cs + GC2]
                    else:
                        gap = xg[:]
                        xs = None
                        xg_k = lambda k0, n, off=0: bass.AP(
                            tensor=gap.tensor,
                            offset=gap.offset + cs * 8 + k0,
                            ap=[gap.ap[0], [1, n], [8, GC2]])

                    # double-width: both dirs side by side, shared folds
                    m = p_m.tile([PB, 7, 2 * GC2], bf16, tag="m")
                    pq = p_pq.tile([PB, 6, 2 * GC2], bf16, tag="pq")

                    for d in (0, 1):
                        if roll and d == 0:
                            lo = cs + L
                            if lo + GC2 <= S:
                                xp_ap = xf[:, :, lo:lo + GC2]
                            else:
                                xp = p_xp.tile([PB, 7, GC2], bf16, tag="xp")
                                mn = S - lo
                                nc.vector.tensor_copy(xp[:, :, 0:mn],
                                                      xf[:, :, lo:S])
                                nc.vector.tensor_copy(xp[:, :, mn:GC2],
                                                      xf[:, :, 0:GC2 - mn])
                                xp_ap = xp[:]
                        elif roll and d == 1:
                            xp = p_xp.tile([PB, 7, GC2], bf16, tag="xp")
                            nrow = GC2 // L
                            src = bass.AP(
                                tensor=xf.tensor, offset=xf[:].offset + cs + 1,
                                ap=[xf[:].ap[0], [S, 7], [L, nrow], [1, L - 1]])
                            dst = bass.AP(
                                tensor=xp.tensor, offset=xp[:].offset,
                                ap=[xp[:].ap[0], [GC2, 7], [L, nrow], [1, L - 1]])
                            nc.gpsimd.tensor_copy(dst, src)
                            srcw = bass.AP(
                                tensor=xf.tensor, offset=xf[:].offset + cs,
                                ap=[xf[:].ap[0], [S, 7], [L, nrow]])
                            dstw = bass.AP(
                                tensor=xp.tensor, offset=xp[:].offset + L - 1,
                                ap=[xp[:].ap[0], [GC2, 7], [L, nrow]])
                            nc.gpsimd.tensor_copy(dstw, srcw)
                            xp_ap = xp[:]
                        else:
                            xpg = p_xp.tile([PB, GC2, 8], bf16, tag="xp")
                            nc.gpsimd.ap_gather(
                                xpg[:], xg[:],
                                idx_sb[d][:, cs // 16:(cs + GC2) // 16],
                                channels=PB, num_elems=S, d=8, num_idxs=GC2)
                            gp = xpg[:]
                            xp_k = lambda k0, n: bass.AP(
                                tensor=gp.tensor, offset=gp.offset + k0,
                                ap=[gp.ap[0], [1, n], [8, GC2]])

                        ms = m[:, :, d * GC2:(d + 1) * GC2]
                        pqs = pq[:, :, d * GC2:(d + 1) * GC2]
                        if roll:
                            nc.vector.tensor_tensor(ms, xs, xp_ap, op=Op.mult)
                            nc.vector.tensor_tensor(
                                pqs[:, 0:3, :], xs[:, 0:3, :],
                                xp_ap[:, 4:7, :], op=Op.mult)
                            nc.vector.tensor_tensor(
                                pqs[:, 3:6, :], xs[:, 4:7, :],
                                xp_ap[:, 0:3, :], op=Op.mult)
                        else:
                            nc.vector.tensor_tensor(ms, xg_k(0, 7),
                                                    xp_k(0, 7), op=Op.mult)
                            nc.vector.tensor_tensor(pqs[:, 0:3, :], xg_k(0, 3),
                                                    xp_k(4, 3), op=Op.mult)
                            nc.vector.tensor_tensor(pqs[:, 3:6, :], xg_k(4, 3),
                                                    xp_k(0, 3), op=Op.mult)

                    # shared folds across both dirs (double width)
                    nc.vector.tensor_tensor(m[:, 0:3, :], m[:, 0:3, :],
                                            m[:, 4:7, :], op=Op.subtract)
                    nc.vector.tensor_tensor(m[:, 0:2, :], m[:, 0:2, :],
                                            m[:, 2:4, :], op=Op.add)
                    nc.vector.tensor_tensor(m[:, 0, :], m[:, 0, :],
                                            m[:, 1, :], op=Op.add)
                    nc.vector.tensor_tensor(pq[:, 0:3, :], pq[:, 0:3, :],
                                            pq[:, 3:6, :], op=Op.add)
                    nc.vector.tensor_tensor(pq[:, 0, :], pq[:, 0, :],
                                            pq[:, 1, :], op=Op.add)
                    nc.vector.tensor_tensor(m[:, 1, :], pq[:, 0, :],
                                            pq[:, 2, :], op=Op.add)

                    # one fused square+accum per chunk (wr,wi x both dirs),
                    # squared in place over the fold results
                    nc.scalar.activation(m[:, 0:2, :], m[:, 0:2, :], Act.Square,
                                         accum_out=acc[:, ch:ch + 1])

                st2.close()

            # ======== final reduce + affine ========
            stot = small_pool.tile([PB, 1], f32)
            nc.vector.tensor_reduce(stot[:], acc[:], axis=X, op=Op.add)
            res = small_pool.tile([PB, 1], f32)
            nc.vector.tensor_scalar(res[:], stot[:], -NBETA, NBETA * 2.0 * S,
                                    op0=Op.mult, op1=Op.add)
            nc.sync.dma_start(out_d[:], res[:])

    nc.finalize()
    return nc



def _build_fast(reps=1, mode="full", dummy_io=False):
    """Roll-shift fast path, instruction-count-minimal.

    This runtime is per-instruction-overhead dominated (~40-100us per
    instruction regardless of size), so the kernel uses FEW, HUGE
    instructions: 2 stage-1 chunks of 2048 sites, and a single full-S
    pass per direction in stage 2 (~65 instructions per rep vs ~75 for
    the previous baseline).

    Layout: x embedding stored angle-major in a 65-padded site layout
    (site (r,c) at p=65r+c; col 64 of each row = col 0 copy; row 64 =
    row 0 copy), so both lattice neighbor shifts are pure offset views
    (+1 for d=1, +65 for d=0) -- no gather/shift copies at all.

    Math (identical to the proven baseline): wrap via k=RNE(phi/2pi),
    phir=phi-2pi*k; sigma=Sign(phir); s_j=sigma*Sin(phir) (j<5),
    c_j=Sin(sigma*pi/2-phir)=sigma*cos(phir) (c_5 fixed up by sigma);
    hyperspherical embedding x via cumprod; per dir:
    m_k=x_k x'_k, pq = zr*zi' / zi*zr'; dre/dim folds; ACT Square+accum.
    """
    import concourse.bass as bass
    import concourse.tile as tile
    from concourse import bacc, mybir

    f32 = mybir.dt.float32
    bf16 = mybir.dt.bfloat16
    i16 = mybir.dt.int16
    Act = mybir.ActivationFunctionType
    Op = mybir.AluOpType
    X = mybir.AxisListType.X

    FC = 2048                 # stage-1 chunk (sites)
    FN = S // FC
    FR = FC // L              # rows per chunk
    M = FC * NA

    nc = bacc.Bacc(None, target_bir_lowering=False)
    phi_kind = "Internal" if dummy_io else "ExternalInput"
    phi_d = nc.dram_tensor("phi", [PB, S, NA], f32, kind=phi_kind)
    out_d = nc.dram_tensor("out", [PB, 1], f32, kind="ExternalOutput")
    pd_flat = phi_d[:].rearrange("p s a -> p (s a)")

    def ap(tile_, off, dims):
        base = tile_[:]
        return bass.AP(tensor=base.tensor, offset=base.offset + off,
                       ap=[base.ap[0]] + dims)

    with tile.TileContext(nc) as tc:
        with contextlib.ExitStack() as ctx:
            xp = ctx.enter_context(tc.tile_pool(name="xp", bufs=1))
            sp = ctx.enter_context(tc.tile_pool(name="sp", bufs=1))

            xf = xp.tile([PB, 7, XLEN], bf16)
            acc = sp.tile([PB, max(reps, 1)], f32)

            for rep in range(reps):
                st1 = contextlib.ExitStack()
                p_phi = st1.enter_context(tc.tile_pool(name="p_phi", bufs=1))
                p_ks = st1.enter_context(tc.tile_pool(name="p_ks", bufs=1))
                p_t = st1.enter_context(tc.tile_pool(name="p_t", bufs=1))
                p_u = st1.enter_context(tc.tile_pool(name="p_u", bufs=1))
                p_cum = st1.enter_context(tc.tile_pool(name="p_cum", bufs=1))

                for ch in range(FN):
                    cs = ch * FC
                    poff = (cs // L) * ROWP

                    phic = p_phi.tile([PB, M], f32, tag="phic")
                    nc.sync.dma_start(phic[:], pd_flat[:, cs * NA:(cs + FC) * NA])
                    if mode == "dma":
                        nc.vector.tensor_reduce(acc[:, 0:1], phic[:, 0:8],
                                                axis=X, op=Op.add)
                        continue

                    # k = RNE(phi/2pi); phir = phi - 2pi*k (in place)
                    k = p_ks.tile([PB, M], i16, tag="ks")
                    nc.vector.tensor_scalar(k[:], phic[:], 1.0 / (2 * PI), None,
                                            op0=Op.mult)
                    nc.vector.scalar_tensor_tensor(
                        phic[:], k[:], -2 * PI, phic[:], op0=Op.mult, op1=Op.add)

                    sig = p_ks.tile([PB, M], bf16, tag="ks")
                    nc.scalar.activation(sig[:], phic[:], Act.Sign)
                    tt_ = p_t.tile([PB, M], bf16, tag="t")
                    nc.scalar.activation(tt_[:], phic[:], Act.Sin)
                    # arg2 = sigma*pi/2 - phir (in place); u = Sin(arg2)
                    nc.vector.scalar_tensor_tensor(
                        phic[:], sig[:], PI / 2, phic[:],
                        op0=Op.mult, op1=Op.subtract)
                    uu = p_u.tile([PB, M], bf16, tag="u")
                    nc.scalar.activation(uu[:], phic[:], Act.Sin)

                    def ang(tile_, j, n=1):
                        a_ = tile_[:]
                        if n == 1:
                            return bass.AP(tensor=a_.tensor,
                                           offset=a_.offset + j,
                                           ap=[a_.ap[0], [NA, FC]])
                        return bass.AP(tensor=a_.tensor, offset=a_.offset + j,
                                       ap=[a_.ap[0], [NA, FC], [1, n]])

                    # s_j = sigma*t (j<5); c_5 = sigma*u (in place)
                    nc.vector.tensor_tensor(ang(tt_, 0, 5), ang(tt_, 0, 5),
                                            ang(sig, 0, 5), op=Op.mult)
                    nc.vector.tensor_tensor(ang(uu, 5), ang(uu, 5),
                                            ang(sig, 5), op=Op.mult)

                    # cumprod + x build into padded xf rows
                    cumA = p_cum.tile([PB, FC], bf16, tag="cumA")
                    cumB = p_cum.tile([PB, FC], bf16, tag="cumB")

                    def xrow(kk):
                        return ap(xf, kk * XLEN + poff, [[ROWP, FR], [1, L]])

                    def angr(tile_, j):
                        a_ = tile_[:]
                        return bass.AP(tensor=a_.tensor, offset=a_.offset + j,
                                       ap=[a_.ap[0], [NA * L, FR], [NA, L]])

                    flat = [[L, FR], [1, L]]
                    TT = nc.vector.tensor_tensor
                    nc.vector.tensor_copy(xrow(0), angr(uu, 0))
                    TT(xrow(1), angr(uu, 1), angr(tt_, 0), op=Op.mult)
                    TT(ap(cumA, 0, flat), angr(tt_, 0), angr(tt_, 1), op=Op.mult)
                    TT(xrow(2), angr(uu, 2), ap(cumA, 0, flat), op=Op.mult)
                    TT(ap(cumB, 0, flat), ap(cumA, 0, flat), angr(tt_, 2),
                       op=Op.mult)
                    TT(xrow(3), angr(uu, 3), ap(cumB, 0, flat), op=Op.mult)
                    TT(ap(cumA, 0, flat), ap(cumB, 0, flat), angr(tt_, 3),
                       op=Op.mult)
                    TT(xrow(4), angr(uu, 4), ap(cumA, 0, flat), op=Op.mult)
                    TT(ap(cumB, 0, flat), ap(cumA, 0, flat), angr(tt_, 4),
                       op=Op.mult)
                    TT(xrow(5), angr(uu, 5), ap(cumB, 0, flat), op=Op.mult)
                    TT(xrow(6), ap(cumB, 0, flat), angr(tt_, 5), op=Op.mult)

                # col pads for all rows at once, then the wrap row
                nc.scalar.copy(
                    ap(xf, L, [[XLEN, 7], [ROWP, NROW]]),
                    ap(xf, 0, [[XLEN, 7], [ROWP, NROW]]))
                nc.scalar.copy(
                    ap(xf, NROW * ROWP, [[XLEN, 7], [1, ROWP]]),
                    ap(xf, 0, [[XLEN, 7], [1, ROWP]]))

                st1.close()
                if mode in ("dma", "stage1"):
                    continue

                # ======== stage 2: one full-S pass per direction ========
                st2 = contextlib.ExitStack()
                p_mt = st2.enter_context(tc.tile_pool(name="p_mt", bufs=1))
                # dir 1 uses rows shifted by +2 so both dirs' dre/dim
                # land in contiguous rows 0-3 and share one Square+accum
                mt = p_mt.tile([PB, 15, S], bf16, tag="mt")

                NR = S // L

                TT = nc.vector.tensor_tensor
                for d, off in ((0, ROWP), (1, 1)):
                    r0 = 2 * d

                    def mrow(i, n=1):
                        return ap(mt, (r0 + i) * S, [[S, n], [L, NR], [1, L]])

                    def mfl(i, n=1):
                        return ap(mt, (r0 + i) * S, [[S, n], [1, S]])

                    def xv(k0, n, o):
                        return ap(xf, k0 * XLEN + o,
                                  [[XLEN, n], [ROWP, NR], [1, L]])

                    # m_k rows 0-6; pq rows 7-9 (zr*zi') and 10-12 (zi*zr')
                    TT(mrow(0, 7), xv(0, 7, 0), xv(0, 7, off), op=Op.mult)
                    TT(mrow(7, 3), xv(0, 3, 0), xv(4, 3, off), op=Op.mult)
                    TT(mrow(10, 3), xv(4, 3, 0), xv(0, 3, off), op=Op.mult)
                    # dre folds: m[0:3]-=m[4:7]; m[0:2]+=m[2:4]
                    TT(mfl(0, 3), mfl(0, 3), mfl(4, 3), op=Op.subtract)
                    TT(mfl(0, 2), mfl(0, 2), mfl(2, 2), op=Op.add)
                    # dim partial: pq[7:10]+=pq[10:13]
                    TT(mfl(7, 3), mfl(7, 3), mfl(10, 3), op=Op.add)
                    # fused: m0+=m1 AND pq7+=pq8 (rows {0,7} += rows {1,8})
                    def mpair(i):
                        return ap(mt, (r0 + i) * S, [[7 * S, 2], [1, S]])
                    TT(mpair(0), mpair(0), mpair(1), op=Op.add)
                    # dim finish: m1 = pq7 + pq9
                    TT(mfl(1), mfl(7), mfl(9), op=Op.add)
                # one Square+accum over rows 0-3 (dre0, dim0, dre1, dim1)
                nc.scalar.activation(
                    ap(mt, 0, [[S, 4], [1, S]]),
                    ap(mt, 0, [[S, 4], [1, S]]),
                    Act.Square, accum_out=acc[:, rep:rep + 1])
                st2.close()

            # ======== final reduce + affine ========
            stot = sp.tile([PB, 1], f32)
            nc.vector.tensor_reduce(stot[:], acc[:, reps - 1:reps],
                                    axis=X, op=Op.add)
            res = sp.tile([PB, 1], f32)
            nc.vector.tensor_scalar(res[:], stot[:], -NBETA, NBETA * 2.0 * S,
                                    op0=Op.mult, op1=Op.add)
            nc.sync.dma_start(out_d[:], res[:])

    nc.finalize()
    return nc


def _build_fast3(reps=1, mode="full", dummy_io=False):
    """Software-pipelined roll-shift fast path (v3).

    Same math and layouts as _build_fast2, but stage-2 quarter q of rep
    i-1 is EMITTED just before stage-1 chunk q of rep i. Engine streams
    execute in order, so this interleaving lets the ACT trig of the next
    rep run underneath the DVE product/fold work of the current one; the
    xf buffer rotates at quarter granularity via region-level WAR deps
    (chunk q == quarter q), with the wrap row copied late so stage-2's
    torus reads never block the refill. Single-buffered stage-1 tiles +
    one mt buffer keep the concurrent working set under the SBUF limit.
    """
    import concourse.bass as bass
    import concourse.tile as tile
    from concourse import bacc, mybir

    f32 = mybir.dt.float32
    bf16 = mybir.dt.bfloat16
    i16 = mybir.dt.int16
    Act = mybir.ActivationFunctionType
    Op = mybir.AluOpType
    X = mybir.AxisListType.X

    FC = 1024                 # chunk == quarter (sites)
    CN = S // FC
    FR = FC // L
    M = FC * NA
    DS = 2 * FC               # both dirs side by side

    nc = bacc.Bacc(None, target_bir_lowering=False)
    phi_kind = "Internal" if dummy_io else "ExternalInput"
    phi_d = nc.dram_tensor("phi", [PB, S, NA], f32, kind=phi_kind)
    out_d = nc.dram_tensor("out", [PB, 1], f32, kind="ExternalOutput")
    pd_flat = phi_d[:].rearrange("p s a -> p (s a)")

    def ap(tile_, off, dims):
        base = tile_[:]
        return bass.AP(tensor=base.tensor, offset=base.offset + off,
                       ap=[base.ap[0]] + dims)

    with tile.TileContext(nc) as tc:
        with contextlib.ExitStack() as ctx:
            xp = ctx.enter_context(tc.tile_pool(name="xp", bufs=1))
            sp = ctx.enter_context(tc.tile_pool(name="sp", bufs=1))
            php = ctx.enter_context(tc.tile_pool(name="php", bufs=1))
            p_phr = ctx.enter_context(tc.tile_pool(name="p_phr", bufs=1))
            p_sig = ctx.enter_context(tc.tile_pool(name="p_sig", bufs=1))
            p_t = ctx.enter_context(tc.tile_pool(name="p_t", bufs=1))
            p_u = ctx.enter_context(tc.tile_pool(name="p_u", bufs=1))
            p_cb = ctx.enter_context(tc.tile_pool(name="p_cb", bufs=1))
            p_mt = ctx.enter_context(tc.tile_pool(name="p_mt", bufs=1))

            xf = xp.tile([PB, 7, XLEN], bf16)
            acc = sp.tile([PB, CN], f32)
            cpi2 = sp.tile([PB, 1], f32)
            nc.vector.memset(cpi2[:], PI / 2)
            cb = p_cb.tile([PB, 7, FC], bf16, tag="cb")
            nc.vector.memset(cb[:, 0, :], 1.0)
            mt = p_mt.tile([PB, 13, DS], bf16, tag="mt")

            def dma(ch):
                phic = php.tile([PB, M], f32, tag="phic")
                cs = ch * FC
                nc.sync.dma_start(phic[:], pd_flat[:, cs * NA:(cs + FC) * NA])
                return phic

            def wrap(phic):
                # k = RNE(phi/2pi); phir = phi - 2pi*k, deinterleaved to
                # bf16 rows. k shares the sig tile's allocation (its
                # lifetime ends exactly when sig is written).
                js = [[1, NA], [NA, FC]]
                k = p_sig.tile([PB, M], i16, tag="ks")
                nc.vector.tensor_scalar(k[:], phic[:], 1.0 / (2 * PI), None,
                                        op0=Op.mult)
                phr = p_phr.tile([PB, NA, FC], bf16, tag="phr")
                nc.vector.scalar_tensor_tensor(
                    ap(phr, 0, [[FC, NA], [1, FC]]),
                    ap(k, 0, js), -2 * PI, ap(phic, 0, js),
                    op0=Op.mult, op1=Op.add)
                return phr

            def trig(phr):
                tt = p_t.tile([PB, NA, FC], bf16, tag="t")
                nc.scalar.activation(tt[:], phr[:], Act.Sin)
                sig = p_sig.tile([PB, NA, FC], bf16, tag="ks")
                nc.scalar.activation(sig[:, 0:5, :], phr[:, 0:5, :], Act.Sign)
                nc.scalar.activation(phr[:], phr[:], Act.Abs)
                uu = p_u.tile([PB, NA, FC], bf16, tag="u")
                nc.scalar.activation(uu[:], phr[:], Act.Sin,
                                     bias=cpi2[:], scale=-1.0)
                return tt, sig, uu

            def xbuild(ch, tt, sig, uu):
                poff = ch * FR * ROWP
                nc.vector.tensor_tensor(tt[:, 0:5, :], tt[:, 0:5, :],
                                        sig[:, 0:5, :], op=Op.mult)
                nc.vector.tensor_tensor(uu[:, 0:5, :], uu[:, 0:5, :],
                                        sig[:, 0:5, :], op=Op.mult)
                # cumprod tree: CB rows [1,C1,C2,C3,C4,C5,Q45]
                nc.scalar.copy(cb[:, 1, :], tt[:, 0, :])
                nc.vector.tensor_tensor(
                    ap(cb, 2 * FC, [[2 * FC, 3], [1, FC]]),
                    ap(tt, 0, [[2 * FC, 3], [1, FC]]),
                    ap(tt, FC, [[2 * FC, 3], [1, FC]]), op=Op.mult)
                nc.vector.tensor_tensor(cb[:, 4, :], cb[:, 4, :],
                                        cb[:, 2, :], op=Op.mult)
                nc.vector.tensor_tensor(
                    ap(cb, 3 * FC, [[2 * FC, 2], [1, FC]]),
                    ap(cb, 2 * FC, [[2 * FC, 2], [1, FC]]),
                    ap(tt, 2 * FC, [[2 * FC, 2], [1, FC]]), op=Op.mult)

                def xrow(kk_, n=1):
                    return ap(xf, kk_ * XLEN + poff,
                              [[XLEN, n], [ROWP, FR], [1, L]])

                rowsFR = [[L, FR], [1, L]]
                nc.vector.tensor_tensor(
                    xrow(0, 6),
                    ap(uu, 0, [[FC, 6]] + rowsFR),
                    ap(cb, 0, [[FC, 6]] + rowsFR), op=Op.mult)
                nc.vector.tensor_tensor(
                    xrow(6), ap(cb, 4 * FC, rowsFR),
                    ap(cb, 6 * FC, rowsFR), op=Op.mult)
                # per-chunk col pad: col 64 = col 0 for this chunk's rows
                nc.scalar.copy(
                    ap(xf, poff + L, [[XLEN, 7], [ROWP, FR]]),
                    ap(xf, poff, [[XLEN, 7], [ROWP, FR]]))

            def wraprow():
                nc.scalar.copy(ap(xf, NROW * ROWP, [[XLEN, 7], [1, ROWP]]),
                               ap(xf, 0, [[XLEN, 7], [1, ROWP]]))

            def prod_folds(q):
                o = q * FR * ROWP

                def xv(k0, n, extra=0):
                    return ap(xf, k0 * XLEN + o + extra,
                              [[XLEN, n], [ROWP, FR], [1, L]])

                def mv(r0, n, d):
                    return ap(mt, r0 * DS + d * FC,
                              [[DS, n], [L, FR], [1, L]])

                for d, off in ((0, ROWP), (1, 1)):
                    nc.vector.tensor_tensor(mv(0, 7, d), xv(0, 7),
                                            xv(0, 7, off), op=Op.mult)
                    nc.vector.tensor_tensor(mv(7, 3, d), xv(0, 3),
                                            xv(4, 3, off), op=Op.mult)
                    nc.vector.tensor_tensor(mv(10, 3, d), xv(4, 3),
                                            xv(0, 3, off), op=Op.mult)

                def mf(r0, n, stride=None):
                    return ap(mt, r0 * DS, [[stride or DS, n], [1, DS]])

                nc.vector.tensor_tensor(mf(0, 3), mf(0, 3), mf(4, 3),
                                        op=Op.subtract)
                nc.vector.tensor_tensor(mf(0, 2), mf(0, 2), mf(2, 2),
                                        op=Op.add)
                nc.vector.tensor_tensor(mf(7, 3), mf(7, 3), mf(10, 3),
                                        op=Op.add)
                nc.vector.tensor_tensor(mf(0, 2, 7 * DS), mf(0, 2, 7 * DS),
                                        mf(1, 2, 7 * DS), op=Op.add)
                nc.vector.tensor_tensor(mf(1, 1), mf(7, 1), mf(9, 1),
                                        op=Op.add)

            def square(q):
                nc.scalar.activation(ap(mt, 0, [[DS, 2], [1, DS]]),
                                     ap(mt, 0, [[DS, 2], [1, DS]]),
                                     Act.Square, accum_out=acc[:, q:q + 1])

            # ---- software-pipelined emission ----
            # DVE stream per position: wrap(q) | PF(prev q) | xbuild(q);
            # ACT trig(q) runs under PF(prev q); SQ(prev q) after trig.
            phic = dma(0)
            for rep in range(reps):
                for q in range(CN):
                    phr = wrap(phic)
                    if q + 1 < CN or rep + 1 < reps:
                        phic = dma((q + 1) % CN)      # prefetch next chunk
                    tsu = trig(phr)
                    if rep > 0:
                        prod_folds(q)
                    xbuild(q, *tsu)
                    if rep > 0:
                        square(q)
                wraprow()
            for q in range(CN):
                prod_folds(q)
                square(q)

            stot = sp.tile([PB, 1], f32)
            nc.vector.tensor_reduce(stot[:], acc[:], axis=X, op=Op.add)
            res = sp.tile([PB, 1], f32)
            nc.vector.tensor_scalar(res[:], stot[:], -NBETA, NBETA * 2.0 * S,
                                    op0=Op.mult, op1=Op.add)
            nc.sync.dma_start(out_d[:], res[:])

    nc.finalize()
    return nc


def _build_fast2(reps=1, mode="full", dummy_io=False,
                 pool_folds=False, cp_act=True,
                 phr_bufs=2, mt_bufs=2, php_bufs=1, wrap_arw=True,
                 abs_cos=True):
    """Restructured roll-shift fast path (v2).

    vs _build_fast: angle-major (de-interleaved) trig outputs so all DVE
    ops are row-contiguous; pair-tree cumprod (4 ops instead of a 5-long
    serial chain); both lattice directions packed side by side in each
    stage-2 row so every fold/square instruction covers them at once;
    copies/pads and two of the folds moved to the otherwise-idle gpsimd
    engine; phi pool double-buffered so chunk DMAs overlap compute.

    Math identical to _build_fast: wrap via k=RNE(phi/2pi),
    phir=phi-2pi*k; sigma=Sign(phir); s~=sigma*sin(phir) (|sin|, j<5);
    u=Sin(sigma*pi/2-phir)=sigma*cos(phir) (c5 fixed by extra sigma5);
    hyperspherical x via cumprod tree; per dir m_k/pq products; fold
    tree; Square+accum.
    """
    import concourse.bass as bass
    import concourse.tile as tile
    from concourse import bacc, mybir

    f32 = mybir.dt.float32
    bf16 = mybir.dt.bfloat16
    i16 = mybir.dt.int16
    Act = mybir.ActivationFunctionType
    Op = mybir.AluOpType
    X = mybir.AxisListType.X

    FC = 1024                 # stage-1 chunk (sites)
    CN = S // FC              # 4 chunks
    FR = FC // L              # 16 lattice rows per chunk
    M = FC * NA               # 6144 angles per chunk
    QS = 1024                 # stage-2 quarter (sites per dir)
    NQ = S // QS
    NRQ = QS // L             # 16 lattice rows per quarter
    DS = 2 * QS               # both dirs side by side

    nc = bacc.Bacc(None, target_bir_lowering=False)
    phi_kind = "Internal" if dummy_io else "ExternalInput"
    phi_d = nc.dram_tensor("phi", [PB, S, NA], f32, kind=phi_kind)
    out_d = nc.dram_tensor("out", [PB, 1], f32, kind="ExternalOutput")
    pd_flat = phi_d[:].rearrange("p s a -> p (s a)")

    def ap(tile_, off, dims):
        base = tile_[:]
        return bass.AP(tensor=base.tensor, offset=base.offset + off,
                       ap=[base.ap[0]] + dims)

    with tile.TileContext(nc) as tc:
        with contextlib.ExitStack() as ctx:
            xp = ctx.enter_context(tc.tile_pool(name="xp", bufs=1))
            sp = ctx.enter_context(tc.tile_pool(name="sp", bufs=1))
            php = ctx.enter_context(tc.tile_pool(name="php", bufs=php_bufs))

            xf = xp.tile([PB, 7, XLEN], bf16)
            acc = sp.tile([PB, NQ], f32)
            cpi2 = sp.tile([PB, 1], f32)
            nc.vector.memset(cpi2[:], PI / 2)

            fd_eng = nc.gpsimd if pool_folds else nc.vector

            for rep in range(reps):
                st1 = contextlib.ExitStack()
                p_k = st1.enter_context(tc.tile_pool(name="p_k", bufs=1))
                p_phr = st1.enter_context(tc.tile_pool(name="p_phr",
                                                       bufs=phr_bufs))
                p_sig = st1.enter_context(tc.tile_pool(name="p_sig", bufs=1))
                p_t = st1.enter_context(tc.tile_pool(name="p_t", bufs=2))
                p_u = st1.enter_context(tc.tile_pool(name="p_u", bufs=2))
                p_cb = st1.enter_context(tc.tile_pool(name="p_cb", bufs=1))

                # CB row0 = ones so x0..x5 fold into one multiply
                cb = p_cb.tile([PB, 7, FC], bf16, tag="cb")
                nc.vector.memset(cb[:, 0, :], 1.0)

                for ch in range(CN):
                    cs = ch * FC
                    poff = ch * FR * ROWP

                    phic = php.tile([PB, M], f32, tag="phic")
                    nc.sync.dma_start(phic[:], pd_flat[:, cs * NA:(cs + FC) * NA])
                    if mode == "dma":
                        nc.vector.tensor_reduce(acc[:, 0:1], phic[:, 0:8],
                                                axis=X, op=Op.add)
                        continue

                    # phir = phi wrapped to (-pi, pi], written as bf16
                    # ANGLE-MAJOR ROWS so every downstream op is contiguous
                    js = [[1, NA], [NA, FC]]      # (angle, site) on interleaved
                    phr = p_phr.tile([PB, NA, FC], bf16, tag="phr")
                    if wrap_arw:
                        # two single-period wraps cover |phi| < 5pi
                        w1 = p_k.tile([PB, M], f32, tag="k")
                        nc.vector.add_range_wrap(w1[:], phic[:], 0.0, PI, 2 * PI)
                        nc.vector.add_range_wrap(
                            ap(phr, 0, [[FC, NA], [1, FC]]),
                            ap(w1, 0, js), 0.0, PI, 2 * PI)
                    else:
                        # k = RNE(phi/2pi); phir = phi - 2pi*k
                        k = p_k.tile([PB, M], i16, tag="k")
                        nc.vector.tensor_scalar(k[:], phic[:], 1.0 / (2 * PI),
                                                None, op0=Op.mult)
                        nc.vector.scalar_tensor_tensor(
                            ap(phr, 0, [[FC, NA], [1, FC]]),
                            ap(k, 0, js), -2 * PI, ap(phic, 0, js),
                            op0=Op.mult, op1=Op.add)

                    tt = p_t.tile([PB, NA, FC], bf16, tag="t")
                    nc.scalar.activation(tt[:], phr[:], Act.Sin)
                    uu = p_u.tile([PB, NA, FC], bf16, tag="u")
                    if abs_cos:
                        # sigma rows 0..4 only; u = cos(phir) via
                        # Sin(pi/2 - |phir|) (cos is even), c5 free
                        sig = p_sig.tile([PB, 5, FC], bf16, tag="sig")
                        nc.scalar.activation(sig[:], phr[:, 0:5, :], Act.Sign)
                        nc.scalar.activation(phr[:], phr[:], Act.Abs)
                        nc.scalar.activation(uu[:], phr[:], Act.Sin,
                                             bias=cpi2[:], scale=-1.0)
                        # s~_j = sigma*t ; c~_j = sigma*u (j<5, in place)
                        nc.vector.tensor_tensor(tt[:, 0:5, :], tt[:, 0:5, :],
                                                sig[:], op=Op.mult)
                        nc.vector.tensor_tensor(uu[:, 0:5, :], uu[:, 0:5, :],
                                                sig[:], op=Op.mult)
                    else:
                        sig = p_sig.tile([PB, NA, FC], bf16, tag="sig")
                        nc.scalar.activation(sig[:], phr[:], Act.Sign)
                        # arg2 = sigma*pi/2 - phir (in place); u = Sin(arg2)
                        nc.vector.scalar_tensor_tensor(
                            phr[:], sig[:], PI / 2, phr[:],
                            op0=Op.mult, op1=Op.subtract)
                        nc.scalar.activation(uu[:], phr[:], Act.Sin)
                        # s~_j = sigma*t (j<5); c5 = sigma5*u5
                        nc.vector.tensor_tensor(tt[:, 0:5, :], tt[:, 0:5, :],
                                                sig[:, 0:5, :], op=Op.mult)
                        nc.vector.tensor_tensor(uu[:, 5, :], uu[:, 5, :],
                                                sig[:, 5, :], op=Op.mult)

                    # cumprod tree: CB rows [1,C1,C2,C3,C4,C5,Q45]
                    if cp_act:
                        nc.scalar.copy(cb[:, 1, :], tt[:, 0, :])
                    else:
                        nc.vector.tensor_copy(cb[:, 1, :], tt[:, 0, :])
                    # P: rows {2,4,6} = t{0,2,4} * t{1,3,5}
                    nc.vector.tensor_tensor(
                        ap(cb, 2 * FC, [[2 * FC, 3], [1, FC]]),
                        ap(tt, 0, [[2 * FC, 3], [1, FC]]),
                        ap(tt, FC, [[2 * FC, 3], [1, FC]]), op=Op.mult)
                    # C4: row4 *= row2
                    nc.vector.tensor_tensor(cb[:, 4, :], cb[:, 4, :],
                                            cb[:, 2, :], op=Op.mult)
                    # C3,C5: rows {3,5} = rows {2,4} * t{2,4}
                    nc.vector.tensor_tensor(
                        ap(cb, 3 * FC, [[2 * FC, 2], [1, FC]]),
                        ap(cb, 2 * FC, [[2 * FC, 2], [1, FC]]),
                        ap(tt, 2 * FC, [[2 * FC, 2], [1, FC]]), op=Op.mult)

                    def xrow(kk_, n=1):
                        return ap(xf, kk_ * XLEN + poff,
                                  [[XLEN, n], [ROWP, FR], [1, L]])

                    rowsFR = [[L, FR], [1, L]]
                    # x0..x5 = u rows0..5 * CB rows0..5 ; x6 = C4 * Q45
                    nc.vector.tensor_tensor(
                        xrow(0, 6),
                        ap(uu, 0, [[FC, 6]] + rowsFR),
                        ap(cb, 0, [[FC, 6]] + rowsFR), op=Op.mult)
                    nc.vector.tensor_tensor(
                        xrow(6), ap(cb, 4 * FC, rowsFR),
                        ap(cb, 6 * FC, rowsFR), op=Op.mult)

                if mode == "dma":
                    st1.close()
                    continue

                # pads: col 64 = col 0 per row; row 64 = row 0
                pad_cp = nc.scalar.copy if cp_act else nc.vector.tensor_copy
                pad_cp(ap(xf, L, [[XLEN, 7], [ROWP, NROW]]),
                       ap(xf, 0, [[XLEN, 7], [ROWP, NROW]]))
                pad_cp(ap(xf, NROW * ROWP, [[XLEN, 7], [1, ROWP]]),
                       ap(xf, 0, [[XLEN, 7], [1, ROWP]]))

                st1.close()
                if mode == "stage1":
                    nc.vector.tensor_reduce(acc[:, 0:1], xf[:, 0, 0:8],
                                            axis=X, op=Op.add)
                    continue

                # ======== stage 2: quarters, both dirs side by side ========
                st2 = contextlib.ExitStack()
                p_mt = st2.enter_context(tc.tile_pool(name="p_mt",
                                                      bufs=mt_bufs))

                for q in range(NQ):
                    mt = p_mt.tile([PB, 13, DS], bf16, tag="mt")
                    o = q * NRQ * ROWP

                    def xv(k0, n, extra=0):
                        return ap(xf, k0 * XLEN + o + extra,
                                  [[XLEN, n], [ROWP, NRQ], [1, L]])

                    def mv(r0, n, d):
                        return ap(mt, r0 * DS + d * QS,
                                  [[DS, n], [L, NRQ], [1, L]])

                    for d, off in ((0, ROWP), (1, 1)):
                        # m_k rows 0-6; pq rows 7-9 (zr*zi'), 10-12 (zi*zr')
                        nc.vector.tensor_tensor(mv(0, 7, d), xv(0, 7),
                                                xv(0, 7, off), op=Op.mult)
                        nc.vector.tensor_tensor(mv(7, 3, d), xv(0, 3),
                                                xv(4, 3, off), op=Op.mult)
                        nc.vector.tensor_tensor(mv(10, 3, d), xv(4, 3),
                                                xv(0, 3, off), op=Op.mult)

                    def mf(r0, n, stride=None):
                        return ap(mt, r0 * DS, [[stride or DS, n], [1, DS]])

                    # dre: rows0:3 -= rows4:7; rows{0,1} += rows{2,3}
                    nc.vector.tensor_tensor(mf(0, 3), mf(0, 3), mf(4, 3),
                                            op=Op.subtract)
                    nc.vector.tensor_tensor(mf(0, 2), mf(0, 2), mf(2, 2),
                                            op=Op.add)
                    # dim partial: rows7:10 += rows10:13
                    fd_eng.tensor_tensor(mf(7, 3), mf(7, 3), mf(10, 3),
                                         op=Op.add)
                    # fused: row0 += row1 AND row7 += row8
                    nc.vector.tensor_tensor(mf(0, 2, 7 * DS), mf(0, 2, 7 * DS),
                                            mf(1, 2, 7 * DS), op=Op.add)
                    # dim: row1 = row7 + row9
                    fd_eng.tensor_tensor(mf(1, 1), mf(7, 1), mf(9, 1),
                                         op=Op.add)
                    # Square+accum over rows {0,1} (dre, dim; both dirs)
                    nc.scalar.activation(mf(0, 2), mf(0, 2), Act.Square,
                                         accum_out=acc[:, q:q + 1])
                st2.close()

            # ======== final reduce + affine ========
            acc_src = acc[:] if mode == "full" else acc[:, 0:1]
            stot = sp.tile([PB, 1], f32)
            nc.vector.tensor_reduce(stot[:], acc_src, axis=X, op=Op.add)
            res = sp.tile([PB, 1], f32)
            nc.vector.tensor_scalar(res[:], stot[:], -NBETA, NBETA * 2.0 * S,
                                    op0=Op.mult, op1=Op.add)
            nc.sync.dma_start(out_d[:], res[:])

    nc.finalize()
    return nc


def kernel(phi, shift):
    from concourse.bass_utils import run_bass_kernel_spmd

    phi = np.ascontiguousarray(np.asarray(phi, dtype=np.float32))
    shift = np.asarray(shift, dtype=np.int32)
    key = (shift.tobytes(), 1)
    if key not in _cache:
        _cache[key] = _build(shift)
    nc = _cache[key]

    in_maps = [{"phi": phi[i * PB:(i + 1) * PB]} for i in range(NCORES)]
    res = run_bass_kernel_spmd(nc, in_maps, core_ids=list(range(NCORES)))
    out = np.concatenate([r["out"] for r in res.results], axis=0)
    return out.astype(np.float32)

